# revision 28
# baseline (speedup 1.0000x reference)
"""Trainium2 Bass kernel for 3-NN IDW interpolation — spatially pruned version.

Host prep (cheap numpy): per core, queries are kd-ordered into 256 tiles of
128 spatially-coherent queries. Each tile gets a provably sufficient
candidate set (union of sub-cluster bounds: any reference that could be a
top-3 neighbor of any query in the tile). Tiles are processed sorted by
candidate count descending; the SPMD program's per-slot candidate width is
the max across the 8 cores at that slot rank.

Device per tile: bf16-split matmul P = 2*dot - sq2 over candidate columns;
DVE max8 + max_index; GPSIMD shared-column gather of the tile-local f2
table (mod-16 diagonal + E mask); batched tail; PE-transposed output.
Host unpermutes the output.
"""
import sys, os
sys.path.insert(0, '/opt/trn_rl_repo')

import numpy as np
import ml_dtypes
from contextlib import ExitStack

import concourse.bass as bass
import concourse.bacc as bacc
import concourse.tile as tile
from concourse import mybir
from concourse.bass_utils import run_bass_kernel_spmd

F32 = mybir.dt.float32
BF16 = mybir.dt.bfloat16
U16 = mybir.dt.uint16
I32 = mybir.dt.int32
AX = mybir.AxisListType
OP = mybir.AluOpType
ACTF = mybir.ActivationFunctionType

B, N, S = 4, 65536, 512
N_CORES = 8
NQ = N // 2
TQ = 128
NT = NQ // TQ
GRP = 16
EPS = 1e-8
KR = 21
SUB = 16
PAD = 32

_cache = {}
TRACE = False


def build_nc(slot_cands, goff, gspan_max):
    """slot_cands: per-slot candidate width; goff[g] = column offset of group g
    in the packed tables; gspan_max = max group span (pool sizing)."""
    nc = bacc.Bacc("TRN2", target_bir_lowering=False, debug=False,
                   num_devices=N_CORES)
    tot = int(goff[-1])
    lhs_d = nc.dram_tensor("lhs_d", [KR, NQ], BF16, kind="ExternalInput").ap()
    rhs_d = nc.dram_tensor("rhs_d", [KR, tot], BF16, kind="ExternalInput").ap()
    f2_d = nc.dram_tensor("f2_d", [1, tot], F32, kind="ExternalInput").ap()
    sq1_d = nc.dram_tensor("sq1_d", [128, NT], F32, kind="ExternalInput").ap()
    outc = nc.dram_tensor("outc", [NQ], F32, kind="ExternalOutput").ap()
    out2d = outc.rearrange("(t p) -> t p", p=TQ)

    with tile.TileContext(nc) as tc, ExitStack() as ctx:
        const = ctx.enter_context(tc.tile_pool(name="const", bufs=1))
        setup = ctx.enter_context(tc.tile_pool(name="setup", bufs=1))
        lt_pool = ctx.enter_context(tc.tile_pool(name="lt", bufs=3))
        rt_pool = ctx.enter_context(tc.tile_pool(name="rt", bufs=3))
        f2_pool = ctx.enter_context(tc.tile_pool(name="f2p", bufs=3))
        ps_P = ctx.enter_context(tc.tile_pool(name="psP", bufs=6, space="PSUM"))
        ps_T = ctx.enter_context(tc.tile_pool(name="psT", bufs=1, space="PSUM"))
        grp_pool = ctx.enter_context(tc.tile_pool(name="grp", bufs=6))
        tail_pool = ctx.enter_context(tc.tile_pool(name="tail", bufs=2))
        stage_pool = ctx.enter_context(tc.tile_pool(name="stage", bufs=2))

        # E[p, i] = 1.0 iff (i % 16) == (p % 16)
        ramp = const.tile([128, 48], I32)
        nc.gpsimd.iota(ramp[:], pattern=[[0, 3], [1, 16]], base=0,
                       channel_multiplier=0)
        pid = const.tile([128, 48], I32)
        nc.gpsimd.iota(pid[:], pattern=[[0, 48]], base=0, channel_multiplier=1)
        pmod = const.tile([128, 48], I32)
        nc.vector.tensor_scalar(pmod[:], pid[:], 15, None, op0=OP.bitwise_and)
        E = const.tile([128, 48], F32)
        nc.vector.tensor_tensor(E[:], ramp[:], pmod[:], op=OP.is_equal)

        iot_p = const.tile([128, 128], I32)
        nc.gpsimd.iota(iot_p[:], pattern=[[0, 128]], base=0, channel_multiplier=1)
        iot_f = const.tile([128, 128], I32)
        nc.gpsimd.iota(iot_f[:], pattern=[[1, 128]], base=0, channel_multiplier=0)
        ident = const.tile([128, 128], F32)
        nc.vector.tensor_tensor(ident[:], iot_p[:], iot_f[:], op=OP.is_equal)

        sq1_sb = setup.tile([128, NT], F32)
        nc.sync.dma_start(sq1_sb[:], sq1_d[:])

        coff = [0]
        for t in range(NT):
            coff.append(coff[-1] + slot_cands[t])

        n_grp = NT // GRP
        stage = None
        for g in range(n_grp):
            m8g = grp_pool.tile([128, 8 * GRP], F32, tag="m8g")
            mig = grp_pool.tile([128, 8 * GRP], U16, tag="mig")
            g48g = grp_pool.tile([128, 48 * GRP], F32, tag="g48g")
            if g % (128 // GRP) == 0:
                stage = stage_pool.tile([128, 128], F32, tag="stage")

            o0, o1 = int(goff[g]), int(goff[g + 1])
            gspan = o1 - o0
            lt = lt_pool.tile([KR, TQ * GRP], BF16)
            nc.sync.dma_start(lt[:], lhs_d[:, g * GRP * TQ:(g + 1) * GRP * TQ])
            rtg = rt_pool.tile([KR, gspan_max], BF16, tag="rtg")
            nc.sync.dma_start(rtg[:, 0:gspan], rhs_d[:, o0:o1])
            f2g = f2_pool.tile([128, gspan_max], F32, tag="f2g")
            ck = (gspan + 3) // 4
            for q in range(4):
                a, bb = q * ck, min((q + 1) * ck, gspan)
                if a < bb:
                    nc.sync.dma_start(f2g[:, a:bb],
                                      f2_d[0:1, o0 + a:o0 + bb]
                                      .partition_broadcast(128))

            for j in range(GRP):
                t = g * GRP + j
                cnd = slot_cands[t]
                lo = coff[t] - o0
                pP = ps_P.tile([TQ, 512], F32, tag="pP")
                nc.tensor.matmul(pP[:, 0:cnd], lt[:, j * TQ:(j + 1) * TQ],
                                 rtg[:, lo:lo + cnd], start=True, stop=True)
                nc.vector.max(m8g[:, 8 * j:8 * j + 8], pP[:, 0:cnd])
                nc.vector.max_index(mig[:, 8 * j:8 * j + 8],
                                    m8g[:, 8 * j:8 * j + 8], pP[:, 0:cnd])
                if j > 0:
                    jp = j - 1
                    tp = g * GRP + jp
                    lop = coff[tp] - o0
                    nc.gpsimd.indirect_copy(g48g[:, 48 * jp:48 * jp + 48],
                                            f2g[:, lop:lop + slot_cands[tp]],
                                            mig[:, 8 * jp:8 * jp + 3],
                                            i_know_ap_gather_is_preferred=True)
            jp = GRP - 1
            tp = g * GRP + jp
            lop = coff[tp] - o0
            nc.gpsimd.indirect_copy(g48g[:, 48 * jp:48 * jp + 48],
                                    f2g[:, lop:lop + slot_cands[tp]],
                                    mig[:, 8 * jp:8 * jp + 3],
                                    i_know_ap_gather_is_preferred=True)

            # batched tail
            m3 = m8g[:].rearrange("p (j e) -> p j e", e=8)[:, :, 0:3]
            sq1r = sq1_sb[:, g * GRP:(g + 1) * GRP].unsqueeze(-1) \
                                                   .broadcast_to([128, GRP, 3])
            d3 = tail_pool.tile([128, 3 * GRP], F32, tag="d3")
            d3v = d3[:].rearrange("p (j e) -> p j e", e=3)
            nc.vector.tensor_tensor(d3v, sq1r, m3, op=OP.subtract)
            r = tail_pool.tile([128, 3 * GRP], F32, tag="r")
            nc.vector.reciprocal(r[:], d3[:])
            den = tail_pool.tile([128, GRP], F32, tag="den")
            nc.vector.reduce_sum(den[:], r[:].rearrange("p (j e) -> p j e", e=3),
                                 axis=AX.X)
            r_rep = r[:].rearrange("p (j e) -> p j e", e=3).unsqueeze(-1) \
                        .broadcast_to([128, GRP, 3, 16])
            g4 = g48g[:].rearrange("p (j k q) -> p j k q", k=3, q=16)
            t1 = tail_pool.tile([128, 48 * GRP], F32, tag="t1")
            t1v = t1[:].rearrange("p (j k q) -> p j k q", k=3, q=16)
            nc.vector.tensor_tensor(t1v, g4, r_rep, op=OP.mult)
            e_rep = E[:].unsqueeze(1).broadcast_to([128, GRP, 48])
            t2 = tail_pool.tile([128, 48 * GRP], F32, tag="t2")
            t2v = t2[:].rearrange("p (j i) -> p j i", i=48)
            nc.vector.tensor_tensor(t2v, t1[:].rearrange("p (j i) -> p j i", i=48),
                                    e_rep, op=OP.mult)
            num = tail_pool.tile([128, GRP], F32, tag="num")
            nc.vector.reduce_sum(num[:], t2v, axis=AX.X)
            rden = tail_pool.tile([128, GRP], F32, tag="rden")
            nc.vector.reciprocal(rden[:], den[:])
            outv = tail_pool.tile([128, GRP], F32, tag="outv")
            nc.vector.tensor_tensor(outv[:], num[:], rden[:], op=OP.mult)
            col = (g * GRP) % 128
            nc.scalar.activation(stage[:, col:col + GRP], outv[:], ACTF.Sigmoid,
                                 scale=2.0)

            if (g + 1) % (128 // GRP) == 0:
                blk = (g * GRP) // 128
                pT = ps_T.tile([128, 128], F32)
                nc.tensor.transpose(pT[:], stage[:], ident[:])
                oT = stage_pool.tile([128, 128], F32, tag="oT")
                nc.scalar.copy(oT[:], pT[:])
                nc.sync.dma_start(out2d[blk * 128:(blk + 1) * 128, :], oT[:])

    nc.compile()
    return nc


def _split3(v32):
    h = v32.astype(ml_dtypes.bfloat16)
    rr = (v32 - h.astype(np.float32)).astype(np.float32)
    m = rr.astype(ml_dtypes.bfloat16)
    l = (rr - m.astype(np.float32)).astype(ml_dtypes.bfloat16)
    return h, m, l


def _kd_order(pts, leaf):
    idx = np.arange(len(pts))
    out = []
    stack = [idx]
    while stack:
        ids = stack.pop()
        if len(ids) <= leaf:
            out.append(ids)
            continue
        p = pts[ids]
        ax = int(np.argmax(p.max(0) - p.min(0)))
        k = (len(ids) // 2 // leaf) * leaf
        if k == 0:
            k = len(ids) // 2
        part = np.argpartition(p[:, ax], k)
        stack.append(ids[part[k:]])
        stack.append(ids[part[:k]])
    return np.concatenate(out)


def _prep_core(xyz1h, xyz2b, f2row):
    """Returns dict with order, per-tile candidate index lists, counts."""
    q = xyz1h.T.astype(np.float64)          # [NQ, 3]
    r = xyz2b.T.astype(np.float64)          # [S, 3]
    order = _kd_order(q, SUB)
    qs = q[order]
    nsub = NQ // SUB
    qsub = qs.reshape(nsub, SUB, 3)
    c = qsub.mean(1)
    rho = np.sqrt(((qsub - c[:, None, :]) ** 2).sum(2)).max(1)
    dc = np.sqrt(((c[:, None, :] - r[None, :, :]) ** 2).sum(2))
    d3c = np.partition(dc, 2, axis=1)[:, 2]
    need = dc <= (d3c + 2 * rho + 1e-3)[:, None]
    need_t = need.reshape(NT, TQ // SUB, S).any(1)      # [NT, S]
    counts = need_t.sum(1)
    padded = np.maximum(PAD, ((counts + PAD - 1) // PAD) * PAD)
    tile_rank = np.argsort(-padded, kind='stable')       # slot -> original tile
    return {"order": order, "need_t": need_t, "padded": padded,
            "tile_rank": tile_rank}


def _make_core_inputs(xyz1h, xyz2b, f2row, prep, slot_cands, goff):
    order = prep["order"]
    need_t = prep["need_t"]
    tile_rank = prep["tile_rank"]

    x1 = xyz1h.astype(np.float32)[:, order]              # permuted queries
    x2 = xyz2b.astype(np.float32)
    f2 = f2row.astype(np.float32).reshape(-1)

    xh, xm, xl = {}, {}, {}
    for cc in range(3):
        xh[cc], xm[cc], xl[cc] = _split3(x1[cc])
    yh, ym, yl = {}, {}, {}
    for cc in range(3):
        yh[cc], ym[cc], yl[cc] = _split3((2.0 * x2[cc]).astype(np.float32))
    sq2 = ((x2[0] * x2[0] + x2[1] * x2[1]) + x2[2] * x2[2]).astype(np.float32)
    sh, sm, sl = _split3(-sq2)

    onesq = np.ones(NQ, ml_dtypes.bfloat16)
    lhs_rows, rhs_rows = [], []
    for cc in range(3):
        lhs_rows.append(xh[cc]); rhs_rows.append(yh[cc])
    lhs_rows.append(onesq); rhs_rows.append(sh)
    for cc in range(3):
        lhs_rows.append(xh[cc]); rhs_rows.append(ym[cc])
        lhs_rows.append(xm[cc]); rhs_rows.append(yh[cc])
    lhs_rows.append(onesq); rhs_rows.append(sm)
    for cc in range(3):
        lhs_rows.append(xh[cc]); rhs_rows.append(yl[cc])
        lhs_rows.append(xl[cc]); rhs_rows.append(yh[cc])
        lhs_rows.append(xm[cc]); rhs_rows.append(ym[cc])
    lhs_rows.append(onesq); rhs_rows.append(sl)
    lhs = np.stack(lhs_rows).astype(ml_dtypes.bfloat16)      # [KR, NQ]
    rhs_full = np.stack([np.asarray(rr_, np.float32) for rr_ in rhs_rows]) \
                 .astype(np.float32)                          # [KR, S] fp32 view
    rhs_full_bf = np.stack(rhs_rows).astype(ml_dtypes.bfloat16)

    tot = int(goff[-1])
    rhs_pack = np.zeros((KR, tot), ml_dtypes.bfloat16)
    f2_pack = np.zeros((1, tot), np.float32)
    SQ2H_ROW = 3   # the sh row index
    off = 0
    for slot in range(NT):
        t_orig = tile_rank[slot]
        cidx = np.nonzero(need_t[t_orig])[0]
        w = int(slot_cands[slot])
        assert len(cidx) <= w, (slot, len(cidx), w)
        rhs_pack[:, off:off + len(cidx)] = rhs_full_bf[:, cidx]
        if len(cidx) < w:
            rhs_pack[SQ2H_ROW, off + len(cidx):off + w] = \
                ml_dtypes.bfloat16(-1e30)
        f2_pack[0, off:off + len(cidx)] = f2[cidx]
        off += w
    assert off == tot

    # lhs permuted additionally by tile rank: slot s covers original tile
    # tile_rank[s], i.e. queries order[tile_rank[s]*128 : +128]
    qsel = np.concatenate([np.arange(tile_rank[s] * TQ, tile_rank[s] * TQ + TQ)
                           for s in range(NT)])
    lhs_slot = np.ascontiguousarray(lhs[:, qsel])

    sq1 = ((x1[0] * x1[0] + x1[1] * x1[1]) + x1[2] * x1[2]).astype(np.float32)
    sq1e = (sq1 + np.float32(EPS)).astype(np.float32)[qsel]
    sq1_t = np.ascontiguousarray(sq1e.reshape(NT, TQ).T)

    # final query permutation: device position i corresponds to
    # original query order[qsel[i]]
    perm = order[qsel]

    return {
        "lhs_d": lhs_slot,
        "rhs_d": np.ascontiguousarray(rhs_pack),
        "f2_d": np.ascontiguousarray(f2_pack),
        "sq1_d": sq1_t,
    }, perm


def kernel(xyz1, xyz2, points2):
    xyz1 = np.ascontiguousarray(np.asarray(xyz1, dtype=np.float32))
    xyz2 = np.ascontiguousarray(np.asarray(xyz2, dtype=np.float32))
    points2 = np.ascontiguousarray(np.asarray(points2, dtype=np.float32))

    preps = []
    for c in range(N_CORES):
        b, h = c // 2, c % 2
        preps.append(_prep_core(xyz1[b][:, h * NQ:(h + 1) * NQ],
                                xyz2[b], points2[b]))
    padded_sorted = np.stack([np.sort(p["padded"])[::-1] for p in preps])
    slot_cands = padded_sorted.max(0).astype(np.int64)       # [NT]
    goff = np.zeros(NT // GRP + 1, np.int64)
    for g in range(NT // GRP):
        goff[g + 1] = goff[g] + slot_cands[g * GRP:(g + 1) * GRP].sum()
    gspan_max = int(max(goff[g + 1] - goff[g] for g in range(NT // GRP)))

    key = tuple(slot_cands.tolist())
    if key not in _cache:
        _cache[key] = build_nc([int(x) for x in slot_cands], goff, gspan_max)
    nc = _cache[key]

    in_maps, perms = [], []
    for c in range(N_CORES):
        b, h = c // 2, c % 2
        im, perm = _make_core_inputs(xyz1[b][:, h * NQ:(h + 1) * NQ],
                                     xyz2[b], points2[b],
                                     preps[c], slot_cands, goff)
        in_maps.append(im)
        perms.append(perm)

    res = run_bass_kernel_spmd(nc, in_maps, core_ids=list(range(N_CORES)),
                               trace=TRACE)
    if TRACE:
        _cache["last_exec_time_ns"] = res.exec_time_ns
    out = np.empty((B, N), dtype=np.float32)
    for c in range(N_CORES):
        b, h = c // 2, c % 2
        seg = np.empty(NQ, np.float32)
        seg[perms[c]] = res.results[c]["outc"]
        out[b, h * NQ:(h + 1) * NQ] = seg
    return out


if __name__ == "__main__":
    rng = np.random.default_rng(0)
    xyz1 = rng.standard_normal((B, 3, N)).astype(np.float32)
    xyz2 = rng.standard_normal((B, 3, S)).astype(np.float32)
    points2 = rng.standard_normal((B, 1, S)).astype(np.float32)
    out = kernel(xyz1, xyz2, points2)
    print(out.shape, out[0, :5])


# revision 29
# speedup vs baseline: 1.0955x; 1.0955x over previous
"""Trainium2 Bass kernel for 3-NN IDW interpolation — spatially pruned version.

Host prep (cheap numpy): per core, queries are kd-ordered into 256 tiles of
128 spatially-coherent queries. Each tile gets a provably sufficient
candidate set (union of sub-cluster bounds: any reference that could be a
top-3 neighbor of any query in the tile). Tiles are processed sorted by
candidate count descending; the SPMD program's per-slot candidate width is
the max across the 8 cores at that slot rank.

Device per tile: bf16-split matmul P = 2*dot - sq2 over candidate columns;
DVE max8 + max_index; GPSIMD shared-column gather of the tile-local f2
table (mod-16 diagonal + E mask); batched tail; PE-transposed output.
Host unpermutes the output.
"""
import sys, os
sys.path.insert(0, '/opt/trn_rl_repo')

import numpy as np
import ml_dtypes
from contextlib import ExitStack

import concourse.bass as bass
import concourse.bacc as bacc
import concourse.tile as tile
from concourse import mybir
from concourse.bass_utils import run_bass_kernel_spmd

F32 = mybir.dt.float32
BF16 = mybir.dt.bfloat16
U16 = mybir.dt.uint16
I32 = mybir.dt.int32
AX = mybir.AxisListType
OP = mybir.AluOpType
ACTF = mybir.ActivationFunctionType

B, N, S = 4, 65536, 512
N_CORES = 8
NQ = N // 2
TQ = 128
NT = NQ // TQ
GRP = 16
EPS = 1e-8
KR = 21
SUB = 16
PAD = 32

_cache = {}
TRACE = False


def build_nc(slot_cands, goff, gspan_max):
    """slot_cands: per-slot candidate width; goff[g] = column offset of group g
    in the packed tables; gspan_max = max group span (pool sizing)."""
    nc = bacc.Bacc("TRN2", target_bir_lowering=False, debug=False,
                   num_devices=N_CORES)
    tot = int(goff[-1])
    lhs_d = nc.dram_tensor("lhs_d", [KR, NQ], BF16, kind="ExternalInput").ap()
    rhs_d = nc.dram_tensor("rhs_d", [KR, tot], BF16, kind="ExternalInput").ap()
    f2_d = nc.dram_tensor("f2_d", [1, tot], F32, kind="ExternalInput").ap()
    sq1_d = nc.dram_tensor("sq1_d", [128, NT], F32, kind="ExternalInput").ap()
    outc = nc.dram_tensor("outc", [NQ], F32, kind="ExternalOutput").ap()
    out2d = outc.rearrange("(t p) -> t p", p=TQ)

    with tile.TileContext(nc) as tc, ExitStack() as ctx:
        const = ctx.enter_context(tc.tile_pool(name="const", bufs=1))
        setup = ctx.enter_context(tc.tile_pool(name="setup", bufs=1))
        lt_pool = ctx.enter_context(tc.tile_pool(name="lt", bufs=3))
        rt_pool = ctx.enter_context(tc.tile_pool(name="rt", bufs=3))
        f2_pool = ctx.enter_context(tc.tile_pool(name="f2p", bufs=3))
        ps_P = ctx.enter_context(tc.tile_pool(name="psP", bufs=6, space="PSUM"))
        ps_T = ctx.enter_context(tc.tile_pool(name="psT", bufs=1, space="PSUM"))
        grp_pool = ctx.enter_context(tc.tile_pool(name="grp", bufs=6))
        tail_pool = ctx.enter_context(tc.tile_pool(name="tail", bufs=2))
        junk_pool = ctx.enter_context(tc.tile_pool(name="junk", bufs=2))
        stage_pool = ctx.enter_context(tc.tile_pool(name="stage", bufs=2))

        # E[p, i] = 1.0 iff (i % 16) == (p % 16)
        ramp = const.tile([128, 48], I32)
        nc.gpsimd.iota(ramp[:], pattern=[[0, 3], [1, 16]], base=0,
                       channel_multiplier=0)
        pid = const.tile([128, 48], I32)
        nc.gpsimd.iota(pid[:], pattern=[[0, 48]], base=0, channel_multiplier=1)
        pmod = const.tile([128, 48], I32)
        nc.vector.tensor_scalar(pmod[:], pid[:], 15, None, op0=OP.bitwise_and)
        E = const.tile([128, 48], F32)
        nc.vector.tensor_tensor(E[:], ramp[:], pmod[:], op=OP.is_equal)

        iot_p = const.tile([128, 128], I32)
        nc.gpsimd.iota(iot_p[:], pattern=[[0, 128]], base=0, channel_multiplier=1)
        iot_f = const.tile([128, 128], I32)
        nc.gpsimd.iota(iot_f[:], pattern=[[1, 128]], base=0, channel_multiplier=0)
        ident = const.tile([128, 128], F32)
        nc.vector.tensor_tensor(ident[:], iot_p[:], iot_f[:], op=OP.is_equal)

        sq1_sb = setup.tile([128, NT], F32)
        nc.sync.dma_start(sq1_sb[:], sq1_d[:])

        coff = [0]
        for t in range(NT):
            coff.append(coff[-1] + slot_cands[t])

        n_grp = NT // GRP
        stage = None
        for g in range(n_grp):
            stt_mode = g >= n_grp - 3
            m8g = grp_pool.tile([128, 8 * GRP], F32, tag="m8g")
            if not stt_mode:
                mig = grp_pool.tile([128, 8 * GRP], U16, tag="mig")
                g48g = grp_pool.tile([128, 48 * GRP], F32, tag="g48g")
            else:
                fg3 = grp_pool.tile([128, 3 * GRP], F32, tag="fg3")
            if g % (128 // GRP) == 0:
                stage = stage_pool.tile([128, 128], F32, tag="stage")

            o0, o1 = int(goff[g]), int(goff[g + 1])
            gspan = o1 - o0
            lt = lt_pool.tile([KR, TQ * GRP], BF16)
            nc.sync.dma_start(lt[:], lhs_d[:, g * GRP * TQ:(g + 1) * GRP * TQ])
            rtg = rt_pool.tile([KR, gspan_max], BF16, tag="rtg")
            nc.sync.dma_start(rtg[:, 0:gspan], rhs_d[:, o0:o1])
            f2g = f2_pool.tile([128, gspan_max], F32, tag="f2g")
            ck = (gspan + 3) // 4
            for q in range(4):
                a, bb = q * ck, min((q + 1) * ck, gspan)
                if a < bb:
                    nc.sync.dma_start(f2g[:, a:bb],
                                      f2_d[0:1, o0 + a:o0 + bb]
                                      .partition_broadcast(128))

            for j in range(GRP):
                t = g * GRP + j
                cnd = slot_cands[t]
                lo = coff[t] - o0
                pP = ps_P.tile([TQ, 512], F32, tag="pP")
                nc.tensor.matmul(pP[:, 0:cnd], lt[:, j * TQ:(j + 1) * TQ],
                                 rtg[:, lo:lo + cnd], start=True, stop=True)
                nc.vector.max(m8g[:, 8 * j:8 * j + 8], pP[:, 0:cnd])
                if stt_mode:
                    junk = junk_pool.tile([128, 512], F32, tag="junk")
                    for k in range(3):
                        nc.vector.scalar_tensor_tensor(
                            junk[:, 0:cnd], pP[:, 0:cnd],
                            m8g[:, 8 * j + k:8 * j + k + 1],
                            f2g[:, lo:lo + cnd],
                            op0=OP.is_equal, op1=OP.mult,
                            accum_out=fg3[:, 3 * j + k:3 * j + k + 1])
                    continue
                nc.vector.max_index(mig[:, 8 * j:8 * j + 8],
                                    m8g[:, 8 * j:8 * j + 8], pP[:, 0:cnd])
                if j > 0:
                    jp = j - 1
                    tp = g * GRP + jp
                    lop = coff[tp] - o0
                    nc.gpsimd.indirect_copy(g48g[:, 48 * jp:48 * jp + 48],
                                            f2g[:, lop:lop + slot_cands[tp]],
                                            mig[:, 8 * jp:8 * jp + 3],
                                            i_know_ap_gather_is_preferred=True)
            if not stt_mode:
                jp = GRP - 1
                tp = g * GRP + jp
                lop = coff[tp] - o0
                nc.gpsimd.indirect_copy(g48g[:, 48 * jp:48 * jp + 48],
                                        f2g[:, lop:lop + slot_cands[tp]],
                                        mig[:, 8 * jp:8 * jp + 3],
                                        i_know_ap_gather_is_preferred=True)

            # batched tail
            m3 = m8g[:].rearrange("p (j e) -> p j e", e=8)[:, :, 0:3]
            sq1r = sq1_sb[:, g * GRP:(g + 1) * GRP].unsqueeze(-1) \
                                                   .broadcast_to([128, GRP, 3])
            d3 = tail_pool.tile([128, 3 * GRP], F32, tag="d3")
            d3v = d3[:].rearrange("p (j e) -> p j e", e=3)
            nc.vector.tensor_tensor(d3v, sq1r, m3, op=OP.subtract)
            r = tail_pool.tile([128, 3 * GRP], F32, tag="r")
            nc.vector.reciprocal(r[:], d3[:])
            den = tail_pool.tile([128, GRP], F32, tag="den")
            nc.vector.reduce_sum(den[:], r[:].rearrange("p (j e) -> p j e", e=3),
                                 axis=AX.X)
            num = tail_pool.tile([128, GRP], F32, tag="num")
            if stt_mode:
                t1s = tail_pool.tile([128, 3 * GRP], F32, tag="t1s")
                nc.vector.tensor_tensor(t1s[:], r[:], fg3[:], op=OP.mult)
                nc.vector.reduce_sum(num[:],
                                     t1s[:].rearrange("p (j e) -> p j e", e=3),
                                     axis=AX.X)
            else:
                r_rep = r[:].rearrange("p (j e) -> p j e", e=3).unsqueeze(-1) \
                            .broadcast_to([128, GRP, 3, 16])
                g4 = g48g[:].rearrange("p (j k q) -> p j k q", k=3, q=16)
                t1 = tail_pool.tile([128, 48 * GRP], F32, tag="t1")
                t1v = t1[:].rearrange("p (j k q) -> p j k q", k=3, q=16)
                nc.vector.tensor_tensor(t1v, g4, r_rep, op=OP.mult)
                e_rep = E[:].unsqueeze(1).broadcast_to([128, GRP, 48])
                t2 = tail_pool.tile([128, 48 * GRP], F32, tag="t2")
                t2v = t2[:].rearrange("p (j i) -> p j i", i=48)
                nc.vector.tensor_tensor(t2v,
                                        t1[:].rearrange("p (j i) -> p j i", i=48),
                                        e_rep, op=OP.mult)
                nc.vector.reduce_sum(num[:], t2v, axis=AX.X)
            rden = tail_pool.tile([128, GRP], F32, tag="rden")
            nc.vector.reciprocal(rden[:], den[:])
            outv = tail_pool.tile([128, GRP], F32, tag="outv")
            nc.vector.tensor_tensor(outv[:], num[:], rden[:], op=OP.mult)
            col = (g * GRP) % 128
            nc.scalar.activation(stage[:, col:col + GRP], outv[:], ACTF.Sigmoid,
                                 scale=2.0)

            if (g + 1) % (128 // GRP) == 0:
                blk = (g * GRP) // 128
                pT = ps_T.tile([128, 128], F32)
                nc.tensor.transpose(pT[:], stage[:], ident[:])
                oT = stage_pool.tile([128, 128], F32, tag="oT")
                nc.scalar.copy(oT[:], pT[:])
                nc.sync.dma_start(out2d[blk * 128:(blk + 1) * 128, :], oT[:])

    nc.compile()
    return nc


def _split3(v32):
    h = v32.astype(ml_dtypes.bfloat16)
    rr = (v32 - h.astype(np.float32)).astype(np.float32)
    m = rr.astype(ml_dtypes.bfloat16)
    l = (rr - m.astype(np.float32)).astype(ml_dtypes.bfloat16)
    return h, m, l


def _kd_order(pts, leaf):
    idx = np.arange(len(pts))
    out = []
    stack = [idx]
    while stack:
        ids = stack.pop()
        if len(ids) <= leaf:
            out.append(ids)
            continue
        p = pts[ids]
        ax = int(np.argmax(p.max(0) - p.min(0)))
        k = (len(ids) // 2 // leaf) * leaf
        if k == 0:
            k = len(ids) // 2
        part = np.argpartition(p[:, ax], k)
        stack.append(ids[part[k:]])
        stack.append(ids[part[:k]])
    return np.concatenate(out)


def _prep_core(xyz1h, xyz2b, f2row):
    """Returns dict with order, per-tile candidate index lists, counts."""
    q = xyz1h.T.astype(np.float64)          # [NQ, 3]
    r = xyz2b.T.astype(np.float64)          # [S, 3]
    order = _kd_order(q, SUB)
    qs = q[order]
    nsub = NQ // SUB
    qsub = qs.reshape(nsub, SUB, 3)
    c = qsub.mean(1)
    rho = np.sqrt(((qsub - c[:, None, :]) ** 2).sum(2)).max(1)
    dc = np.sqrt(((c[:, None, :] - r[None, :, :]) ** 2).sum(2))
    d3c = np.partition(dc, 2, axis=1)[:, 2]
    need = dc <= (d3c + 2 * rho + 1e-3)[:, None]
    need_t = need.reshape(NT, TQ // SUB, S).any(1)      # [NT, S]
    counts = need_t.sum(1)
    padded = np.maximum(PAD, ((counts + PAD - 1) // PAD) * PAD)
    tile_rank = np.argsort(-padded, kind='stable')       # slot -> original tile
    return {"order": order, "need_t": need_t, "padded": padded,
            "tile_rank": tile_rank}


def _make_core_inputs(xyz1h, xyz2b, f2row, prep, slot_cands, goff):
    order = prep["order"]
    need_t = prep["need_t"]
    tile_rank = prep["tile_rank"]

    x1 = xyz1h.astype(np.float32)[:, order]              # permuted queries
    x2 = xyz2b.astype(np.float32)
    f2 = f2row.astype(np.float32).reshape(-1)

    xh, xm, xl = {}, {}, {}
    for cc in range(3):
        xh[cc], xm[cc], xl[cc] = _split3(x1[cc])
    yh, ym, yl = {}, {}, {}
    for cc in range(3):
        yh[cc], ym[cc], yl[cc] = _split3((2.0 * x2[cc]).astype(np.float32))
    sq2 = ((x2[0] * x2[0] + x2[1] * x2[1]) + x2[2] * x2[2]).astype(np.float32)
    sh, sm, sl = _split3(-sq2)

    onesq = np.ones(NQ, ml_dtypes.bfloat16)
    lhs_rows, rhs_rows = [], []
    for cc in range(3):
        lhs_rows.append(xh[cc]); rhs_rows.append(yh[cc])
    lhs_rows.append(onesq); rhs_rows.append(sh)
    for cc in range(3):
        lhs_rows.append(xh[cc]); rhs_rows.append(ym[cc])
        lhs_rows.append(xm[cc]); rhs_rows.append(yh[cc])
    lhs_rows.append(onesq); rhs_rows.append(sm)
    for cc in range(3):
        lhs_rows.append(xh[cc]); rhs_rows.append(yl[cc])
        lhs_rows.append(xl[cc]); rhs_rows.append(yh[cc])
        lhs_rows.append(xm[cc]); rhs_rows.append(ym[cc])
    lhs_rows.append(onesq); rhs_rows.append(sl)
    lhs = np.stack(lhs_rows).astype(ml_dtypes.bfloat16)      # [KR, NQ]
    rhs_full = np.stack([np.asarray(rr_, np.float32) for rr_ in rhs_rows]) \
                 .astype(np.float32)                          # [KR, S] fp32 view
    rhs_full_bf = np.stack(rhs_rows).astype(ml_dtypes.bfloat16)

    tot = int(goff[-1])
    rhs_pack = np.zeros((KR, tot), ml_dtypes.bfloat16)
    f2_pack = np.zeros((1, tot), np.float32)
    SQ2H_ROW = 3   # the sh row index
    off = 0
    for slot in range(NT):
        t_orig = tile_rank[slot]
        cidx = np.nonzero(need_t[t_orig])[0]
        w = int(slot_cands[slot])
        assert len(cidx) <= w, (slot, len(cidx), w)
        rhs_pack[:, off:off + len(cidx)] = rhs_full_bf[:, cidx]
        if len(cidx) < w:
            rhs_pack[SQ2H_ROW, off + len(cidx):off + w] = \
                ml_dtypes.bfloat16(-1e30)
        f2_pack[0, off:off + len(cidx)] = f2[cidx]
        off += w
    assert off == tot

    # lhs permuted additionally by tile rank: slot s covers original tile
    # tile_rank[s], i.e. queries order[tile_rank[s]*128 : +128]
    qsel = np.concatenate([np.arange(tile_rank[s] * TQ, tile_rank[s] * TQ + TQ)
                           for s in range(NT)])
    lhs_slot = np.ascontiguousarray(lhs[:, qsel])

    sq1 = ((x1[0] * x1[0] + x1[1] * x1[1]) + x1[2] * x1[2]).astype(np.float32)
    sq1e = (sq1 + np.float32(EPS)).astype(np.float32)[qsel]
    sq1_t = np.ascontiguousarray(sq1e.reshape(NT, TQ).T)

    # final query permutation: device position i corresponds to
    # original query order[qsel[i]]
    perm = order[qsel]

    return {
        "lhs_d": lhs_slot,
        "rhs_d": np.ascontiguousarray(rhs_pack),
        "f2_d": np.ascontiguousarray(f2_pack),
        "sq1_d": sq1_t,
    }, perm


def kernel(xyz1, xyz2, points2):
    xyz1 = np.ascontiguousarray(np.asarray(xyz1, dtype=np.float32))
    xyz2 = np.ascontiguousarray(np.asarray(xyz2, dtype=np.float32))
    points2 = np.ascontiguousarray(np.asarray(points2, dtype=np.float32))

    preps = []
    for c in range(N_CORES):
        b, h = c // 2, c % 2
        preps.append(_prep_core(xyz1[b][:, h * NQ:(h + 1) * NQ],
                                xyz2[b], points2[b]))
    padded_sorted = np.stack([np.sort(p["padded"])[::-1] for p in preps])
    slot_cands = padded_sorted.max(0).astype(np.int64)       # [NT]
    goff = np.zeros(NT // GRP + 1, np.int64)
    for g in range(NT // GRP):
        goff[g + 1] = goff[g] + slot_cands[g * GRP:(g + 1) * GRP].sum()
    gspan_max = int(max(goff[g + 1] - goff[g] for g in range(NT // GRP)))

    key = tuple(slot_cands.tolist())
    if key not in _cache:
        _cache[key] = build_nc([int(x) for x in slot_cands], goff, gspan_max)
    nc = _cache[key]

    in_maps, perms = [], []
    for c in range(N_CORES):
        b, h = c // 2, c % 2
        im, perm = _make_core_inputs(xyz1[b][:, h * NQ:(h + 1) * NQ],
                                     xyz2[b], points2[b],
                                     preps[c], slot_cands, goff)
        in_maps.append(im)
        perms.append(perm)

    res = run_bass_kernel_spmd(nc, in_maps, core_ids=list(range(N_CORES)),
                               trace=TRACE)
    if TRACE:
        _cache["last_exec_time_ns"] = res.exec_time_ns
    out = np.empty((B, N), dtype=np.float32)
    for c in range(N_CORES):
        b, h = c // 2, c % 2
        seg = np.empty(NQ, np.float32)
        seg[perms[c]] = res.results[c]["outc"]
        out[b, h * NQ:(h + 1) * NQ] = seg
    return out


if __name__ == "__main__":
    rng = np.random.default_rng(0)
    xyz1 = rng.standard_normal((B, 3, N)).astype(np.float32)
    xyz2 = rng.standard_normal((B, 3, S)).astype(np.float32)
    points2 = rng.standard_normal((B, 1, S)).astype(np.float32)
    out = kernel(xyz1, xyz2, points2)
    print(out.shape, out[0, :5])


# revision 30
# speedup vs baseline: 1.1641x; 1.0626x over previous
"""Trainium2 Bass kernel for 3-NN IDW interpolation — spatially pruned version.

Host prep (cheap numpy): per core, queries are kd-ordered into 256 tiles of
128 spatially-coherent queries. Each tile gets a provably sufficient
candidate set (union of sub-cluster bounds: any reference that could be a
top-3 neighbor of any query in the tile). Tiles are processed sorted by
candidate count descending; the SPMD program's per-slot candidate width is
the max across the 8 cores at that slot rank.

Device per tile: bf16-split matmul P = 2*dot - sq2 over candidate columns;
DVE max8 + max_index; GPSIMD shared-column gather of the tile-local f2
table (mod-16 diagonal + E mask); batched tail; PE-transposed output.
Host unpermutes the output.
"""
import sys, os
sys.path.insert(0, '/opt/trn_rl_repo')

import numpy as np
import ml_dtypes
from contextlib import ExitStack

import concourse.bass as bass
import concourse.bacc as bacc
import concourse.tile as tile
from concourse import mybir
from concourse.bass_utils import run_bass_kernel_spmd

F32 = mybir.dt.float32
BF16 = mybir.dt.bfloat16
U16 = mybir.dt.uint16
I32 = mybir.dt.int32
AX = mybir.AxisListType
OP = mybir.AluOpType
ACTF = mybir.ActivationFunctionType

B, N, S = 4, 65536, 512
N_CORES = 8
NQ = N // 2
TQ = 128
NT = NQ // TQ
GRP = 16
EPS = 1e-8
KR = 21
SUB = 16
PAD = 32

_cache = {}
TRACE = False


def build_nc(slot_cands, goff, gspan_max):
    """slot_cands: per-slot candidate width; goff[g] = column offset of group g
    in the packed tables; gspan_max = max group span (pool sizing)."""
    nc = bacc.Bacc("TRN2", target_bir_lowering=False, debug=False,
                   num_devices=N_CORES)
    tot = int(goff[-1])
    lhs_d = nc.dram_tensor("lhs_d", [KR, NQ], BF16, kind="ExternalInput").ap()
    rhs_d = nc.dram_tensor("rhs_d", [KR, tot], BF16, kind="ExternalInput").ap()
    f2_d = nc.dram_tensor("f2_d", [1, tot], F32, kind="ExternalInput").ap()
    sq1_d = nc.dram_tensor("sq1_d", [128, NT], F32, kind="ExternalInput").ap()
    outc = nc.dram_tensor("outc", [NQ], F32, kind="ExternalOutput").ap()
    out2d = outc.rearrange("(t p) -> t p", p=TQ)

    with tile.TileContext(nc) as tc, ExitStack() as ctx:
        const = ctx.enter_context(tc.tile_pool(name="const", bufs=1))
        setup = ctx.enter_context(tc.tile_pool(name="setup", bufs=1))
        lt_pool = ctx.enter_context(tc.tile_pool(name="lt", bufs=3))
        rt_pool = ctx.enter_context(tc.tile_pool(name="rt", bufs=3))
        f2_pool = ctx.enter_context(tc.tile_pool(name="f2p", bufs=3))
        ps_P = ctx.enter_context(tc.tile_pool(name="psP", bufs=6, space="PSUM"))
        ps_T = ctx.enter_context(tc.tile_pool(name="psT", bufs=1, space="PSUM"))
        grp_pool = ctx.enter_context(tc.tile_pool(name="grp", bufs=6))
        tail_pool = ctx.enter_context(tc.tile_pool(name="tail", bufs=2))
        junk_pool = ctx.enter_context(tc.tile_pool(name="junk", bufs=2))
        stage_pool = ctx.enter_context(tc.tile_pool(name="stage", bufs=2))

        # E[p, i] = 1.0 iff (i % 16) == (p % 16)
        ramp = const.tile([128, 48], I32)
        nc.gpsimd.iota(ramp[:], pattern=[[0, 3], [1, 16]], base=0,
                       channel_multiplier=0)
        pid = const.tile([128, 48], I32)
        nc.gpsimd.iota(pid[:], pattern=[[0, 48]], base=0, channel_multiplier=1)
        pmod = const.tile([128, 48], I32)
        nc.vector.tensor_scalar(pmod[:], pid[:], 15, None, op0=OP.bitwise_and)
        E = const.tile([128, 48], F32)
        nc.vector.tensor_tensor(E[:], ramp[:], pmod[:], op=OP.is_equal)

        iot_p = const.tile([128, 128], I32)
        nc.gpsimd.iota(iot_p[:], pattern=[[0, 128]], base=0, channel_multiplier=1)
        iot_f = const.tile([128, 128], I32)
        nc.gpsimd.iota(iot_f[:], pattern=[[1, 128]], base=0, channel_multiplier=0)
        ident = const.tile([128, 128], F32)
        nc.vector.tensor_tensor(ident[:], iot_p[:], iot_f[:], op=OP.is_equal)

        sq1_sb = setup.tile([128, NT], F32)
        nc.sync.dma_start(sq1_sb[:], sq1_d[:])

        coff = [0]
        for t in range(NT):
            coff.append(coff[-1] + slot_cands[t])

        n_grp = NT // GRP
        stage = None
        for g in range(n_grp):
            stt_mode = g >= n_grp - 5
            m8g = grp_pool.tile([128, 8 * GRP], F32, tag="m8g")
            if not stt_mode:
                mig = grp_pool.tile([128, 8 * GRP], U16, tag="mig")
                g48g = grp_pool.tile([128, 48 * GRP], F32, tag="g48g")
            else:
                fg3 = grp_pool.tile([128, 3 * GRP], F32, tag="fg3")
            if g % (128 // GRP) == 0:
                stage = stage_pool.tile([128, 128], F32, tag="stage")

            o0, o1 = int(goff[g]), int(goff[g + 1])
            gspan = o1 - o0
            lt = lt_pool.tile([KR, TQ * GRP], BF16)
            nc.sync.dma_start(lt[:], lhs_d[:, g * GRP * TQ:(g + 1) * GRP * TQ])
            rtg = rt_pool.tile([KR, gspan_max], BF16, tag="rtg")
            nc.sync.dma_start(rtg[:, 0:gspan], rhs_d[:, o0:o1])
            f2g = f2_pool.tile([128, gspan_max], F32, tag="f2g")
            ck = (gspan + 3) // 4
            for q in range(4):
                a, bb = q * ck, min((q + 1) * ck, gspan)
                if a < bb:
                    nc.sync.dma_start(f2g[:, a:bb],
                                      f2_d[0:1, o0 + a:o0 + bb]
                                      .partition_broadcast(128))

            for j in range(GRP):
                t = g * GRP + j
                cnd = slot_cands[t]
                lo = coff[t] - o0
                pP = ps_P.tile([TQ, 512], F32, tag="pP")
                nc.tensor.matmul(pP[:, 0:cnd], lt[:, j * TQ:(j + 1) * TQ],
                                 rtg[:, lo:lo + cnd], start=True, stop=True)
                nc.vector.max(m8g[:, 8 * j:8 * j + 8], pP[:, 0:cnd])
                if stt_mode:
                    junk = junk_pool.tile([128, 512], F32, tag="junk")
                    for k in range(3):
                        nc.vector.scalar_tensor_tensor(
                            junk[:, 0:cnd], pP[:, 0:cnd],
                            m8g[:, 8 * j + k:8 * j + k + 1],
                            f2g[:, lo:lo + cnd],
                            op0=OP.is_equal, op1=OP.mult,
                            accum_out=fg3[:, 3 * j + k:3 * j + k + 1])
                    continue
                nc.vector.max_index(mig[:, 8 * j:8 * j + 8],
                                    m8g[:, 8 * j:8 * j + 8], pP[:, 0:cnd])
                if j > 0:
                    jp = j - 1
                    tp = g * GRP + jp
                    lop = coff[tp] - o0
                    nc.gpsimd.indirect_copy(g48g[:, 48 * jp:48 * jp + 48],
                                            f2g[:, lop:lop + slot_cands[tp]],
                                            mig[:, 8 * jp:8 * jp + 3],
                                            i_know_ap_gather_is_preferred=True)
            if not stt_mode:
                jp = GRP - 1
                tp = g * GRP + jp
                lop = coff[tp] - o0
                nc.gpsimd.indirect_copy(g48g[:, 48 * jp:48 * jp + 48],
                                        f2g[:, lop:lop + slot_cands[tp]],
                                        mig[:, 8 * jp:8 * jp + 3],
                                        i_know_ap_gather_is_preferred=True)

            # batched tail
            m3 = m8g[:].rearrange("p (j e) -> p j e", e=8)[:, :, 0:3]
            sq1r = sq1_sb[:, g * GRP:(g + 1) * GRP].unsqueeze(-1) \
                                                   .broadcast_to([128, GRP, 3])
            d3 = tail_pool.tile([128, 3 * GRP], F32, tag="d3")
            d3v = d3[:].rearrange("p (j e) -> p j e", e=3)
            nc.vector.tensor_tensor(d3v, sq1r, m3, op=OP.subtract)
            r = tail_pool.tile([128, 3 * GRP], F32, tag="r")
            nc.vector.reciprocal(r[:], d3[:])
            den = tail_pool.tile([128, GRP], F32, tag="den")
            nc.vector.reduce_sum(den[:], r[:].rearrange("p (j e) -> p j e", e=3),
                                 axis=AX.X)
            num = tail_pool.tile([128, GRP], F32, tag="num")
            if stt_mode:
                t1s = tail_pool.tile([128, 3 * GRP], F32, tag="t1s")
                nc.vector.tensor_tensor(t1s[:], r[:], fg3[:], op=OP.mult)
                nc.vector.reduce_sum(num[:],
                                     t1s[:].rearrange("p (j e) -> p j e", e=3),
                                     axis=AX.X)
            else:
                r_rep = r[:].rearrange("p (j e) -> p j e", e=3).unsqueeze(-1) \
                            .broadcast_to([128, GRP, 3, 16])
                g4 = g48g[:].rearrange("p (j k q) -> p j k q", k=3, q=16)
                t1 = tail_pool.tile([128, 48 * GRP], F32, tag="t1")
                t1v = t1[:].rearrange("p (j k q) -> p j k q", k=3, q=16)
                nc.vector.tensor_tensor(t1v, g4, r_rep, op=OP.mult)
                e_rep = E[:].unsqueeze(1).broadcast_to([128, GRP, 48])
                t2 = tail_pool.tile([128, 48 * GRP], F32, tag="t2")
                t2v = t2[:].rearrange("p (j i) -> p j i", i=48)
                nc.vector.tensor_tensor(t2v,
                                        t1[:].rearrange("p (j i) -> p j i", i=48),
                                        e_rep, op=OP.mult)
                nc.vector.reduce_sum(num[:], t2v, axis=AX.X)
            rden = tail_pool.tile([128, GRP], F32, tag="rden")
            nc.vector.reciprocal(rden[:], den[:])
            outv = tail_pool.tile([128, GRP], F32, tag="outv")
            nc.vector.tensor_tensor(outv[:], num[:], rden[:], op=OP.mult)
            col = (g * GRP) % 128
            nc.scalar.activation(stage[:, col:col + GRP], outv[:], ACTF.Sigmoid,
                                 scale=2.0)

            if (g + 1) % (128 // GRP) == 0:
                blk = (g * GRP) // 128
                pT = ps_T.tile([128, 128], F32)
                nc.tensor.transpose(pT[:], stage[:], ident[:])
                oT = stage_pool.tile([128, 128], F32, tag="oT")
                nc.scalar.copy(oT[:], pT[:])
                nc.sync.dma_start(out2d[blk * 128:(blk + 1) * 128, :], oT[:])

    nc.compile()
    return nc


def _split3(v32):
    h = v32.astype(ml_dtypes.bfloat16)
    rr = (v32 - h.astype(np.float32)).astype(np.float32)
    m = rr.astype(ml_dtypes.bfloat16)
    l = (rr - m.astype(np.float32)).astype(ml_dtypes.bfloat16)
    return h, m, l


def _kd_order(pts, leaf):
    idx = np.arange(len(pts))
    out = []
    stack = [idx]
    while stack:
        ids = stack.pop()
        if len(ids) <= leaf:
            out.append(ids)
            continue
        p = pts[ids]
        ax = int(np.argmax(p.max(0) - p.min(0)))
        k = (len(ids) // 2 // leaf) * leaf
        if k == 0:
            k = len(ids) // 2
        part = np.argpartition(p[:, ax], k)
        stack.append(ids[part[k:]])
        stack.append(ids[part[:k]])
    return np.concatenate(out)


def _prep_core(xyz1h, xyz2b, f2row):
    """Returns dict with order, per-tile candidate index lists, counts."""
    q = xyz1h.T.astype(np.float64)          # [NQ, 3]
    r = xyz2b.T.astype(np.float64)          # [S, 3]
    order = _kd_order(q, SUB)
    qs = q[order]
    nsub = NQ // SUB
    qsub = qs.reshape(nsub, SUB, 3)
    c = qsub.mean(1)
    rho = np.sqrt(((qsub - c[:, None, :]) ** 2).sum(2)).max(1)
    dc = np.sqrt(((c[:, None, :] - r[None, :, :]) ** 2).sum(2))
    d3c = np.partition(dc, 2, axis=1)[:, 2]
    need = dc <= (d3c + 2 * rho + 1e-3)[:, None]
    need_t = need.reshape(NT, TQ // SUB, S).any(1)      # [NT, S]
    counts = need_t.sum(1)
    padded = np.maximum(PAD, ((counts + PAD - 1) // PAD) * PAD)
    tile_rank = np.argsort(-padded, kind='stable')       # slot -> original tile
    return {"order": order, "need_t": need_t, "padded": padded,
            "tile_rank": tile_rank}


def _make_core_inputs(xyz1h, xyz2b, f2row, prep, slot_cands, goff):
    order = prep["order"]
    need_t = prep["need_t"]
    tile_rank = prep["tile_rank"]

    x1 = xyz1h.astype(np.float32)[:, order]              # permuted queries
    x2 = xyz2b.astype(np.float32)
    f2 = f2row.astype(np.float32).reshape(-1)

    xh, xm, xl = {}, {}, {}
    for cc in range(3):
        xh[cc], xm[cc], xl[cc] = _split3(x1[cc])
    yh, ym, yl = {}, {}, {}
    for cc in range(3):
        yh[cc], ym[cc], yl[cc] = _split3((2.0 * x2[cc]).astype(np.float32))
    sq2 = ((x2[0] * x2[0] + x2[1] * x2[1]) + x2[2] * x2[2]).astype(np.float32)
    sh, sm, sl = _split3(-sq2)

    onesq = np.ones(NQ, ml_dtypes.bfloat16)
    lhs_rows, rhs_rows = [], []
    for cc in range(3):
        lhs_rows.append(xh[cc]); rhs_rows.append(yh[cc])
    lhs_rows.append(onesq); rhs_rows.append(sh)
    for cc in range(3):
        lhs_rows.append(xh[cc]); rhs_rows.append(ym[cc])
        lhs_rows.append(xm[cc]); rhs_rows.append(yh[cc])
    lhs_rows.append(onesq); rhs_rows.append(sm)
    for cc in range(3):
        lhs_rows.append(xh[cc]); rhs_rows.append(yl[cc])
        lhs_rows.append(xl[cc]); rhs_rows.append(yh[cc])
        lhs_rows.append(xm[cc]); rhs_rows.append(ym[cc])
    lhs_rows.append(onesq); rhs_rows.append(sl)
    lhs = np.stack(lhs_rows).astype(ml_dtypes.bfloat16)      # [KR, NQ]
    rhs_full = np.stack([np.asarray(rr_, np.float32) for rr_ in rhs_rows]) \
                 .astype(np.float32)                          # [KR, S] fp32 view
    rhs_full_bf = np.stack(rhs_rows).astype(ml_dtypes.bfloat16)

    tot = int(goff[-1])
    rhs_pack = np.zeros((KR, tot), ml_dtypes.bfloat16)
    f2_pack = np.zeros((1, tot), np.float32)
    SQ2H_ROW = 3   # the sh row index
    off = 0
    for slot in range(NT):
        t_orig = tile_rank[slot]
        cidx = np.nonzero(need_t[t_orig])[0]
        w = int(slot_cands[slot])
        assert len(cidx) <= w, (slot, len(cidx), w)
        rhs_pack[:, off:off + len(cidx)] = rhs_full_bf[:, cidx]
        if len(cidx) < w:
            rhs_pack[SQ2H_ROW, off + len(cidx):off + w] = \
                ml_dtypes.bfloat16(-1e30)
        f2_pack[0, off:off + len(cidx)] = f2[cidx]
        off += w
    assert off == tot

    # lhs permuted additionally by tile rank: slot s covers original tile
    # tile_rank[s], i.e. queries order[tile_rank[s]*128 : +128]
    qsel = np.concatenate([np.arange(tile_rank[s] * TQ, tile_rank[s] * TQ + TQ)
                           for s in range(NT)])
    lhs_slot = np.ascontiguousarray(lhs[:, qsel])

    sq1 = ((x1[0] * x1[0] + x1[1] * x1[1]) + x1[2] * x1[2]).astype(np.float32)
    sq1e = (sq1 + np.float32(EPS)).astype(np.float32)[qsel]
    sq1_t = np.ascontiguousarray(sq1e.reshape(NT, TQ).T)

    # final query permutation: device position i corresponds to
    # original query order[qsel[i]]
    perm = order[qsel]

    return {
        "lhs_d": lhs_slot,
        "rhs_d": np.ascontiguousarray(rhs_pack),
        "f2_d": np.ascontiguousarray(f2_pack),
        "sq1_d": sq1_t,
    }, perm


def kernel(xyz1, xyz2, points2):
    xyz1 = np.ascontiguousarray(np.asarray(xyz1, dtype=np.float32))
    xyz2 = np.ascontiguousarray(np.asarray(xyz2, dtype=np.float32))
    points2 = np.ascontiguousarray(np.asarray(points2, dtype=np.float32))

    preps = []
    for c in range(N_CORES):
        b, h = c // 2, c % 2
        preps.append(_prep_core(xyz1[b][:, h * NQ:(h + 1) * NQ],
                                xyz2[b], points2[b]))
    padded_sorted = np.stack([np.sort(p["padded"])[::-1] for p in preps])
    slot_cands = padded_sorted.max(0).astype(np.int64)       # [NT]
    goff = np.zeros(NT // GRP + 1, np.int64)
    for g in range(NT // GRP):
        goff[g + 1] = goff[g] + slot_cands[g * GRP:(g + 1) * GRP].sum()
    gspan_max = int(max(goff[g + 1] - goff[g] for g in range(NT // GRP)))

    key = tuple(slot_cands.tolist())
    if key not in _cache:
        _cache[key] = build_nc([int(x) for x in slot_cands], goff, gspan_max)
    nc = _cache[key]

    in_maps, perms = [], []
    for c in range(N_CORES):
        b, h = c // 2, c % 2
        im, perm = _make_core_inputs(xyz1[b][:, h * NQ:(h + 1) * NQ],
                                     xyz2[b], points2[b],
                                     preps[c], slot_cands, goff)
        in_maps.append(im)
        perms.append(perm)

    res = run_bass_kernel_spmd(nc, in_maps, core_ids=list(range(N_CORES)),
                               trace=TRACE)
    if TRACE:
        _cache["last_exec_time_ns"] = res.exec_time_ns
    out = np.empty((B, N), dtype=np.float32)
    for c in range(N_CORES):
        b, h = c // 2, c % 2
        seg = np.empty(NQ, np.float32)
        seg[perms[c]] = res.results[c]["outc"]
        out[b, h * NQ:(h + 1) * NQ] = seg
    return out


if __name__ == "__main__":
    rng = np.random.default_rng(0)
    xyz1 = rng.standard_normal((B, 3, N)).astype(np.float32)
    xyz2 = rng.standard_normal((B, 3, S)).astype(np.float32)
    points2 = rng.standard_normal((B, 1, S)).astype(np.float32)
    out = kernel(xyz1, xyz2, points2)
    print(out.shape, out[0, :5])


# revision 31
# speedup vs baseline: 1.2532x; 1.0765x over previous
"""Trainium2 Bass kernel for 3-NN IDW interpolation — spatially pruned version.

Host prep (cheap numpy): per core, queries are kd-ordered into 256 tiles of
128 spatially-coherent queries. Each tile gets a provably sufficient
candidate set (union of sub-cluster bounds: any reference that could be a
top-3 neighbor of any query in the tile). Tiles are processed sorted by
candidate count descending; the SPMD program's per-slot candidate width is
the max across the 8 cores at that slot rank.

Device per tile: bf16-split matmul P = 2*dot - sq2 over candidate columns;
DVE max8 + max_index; GPSIMD shared-column gather of the tile-local f2
table (mod-16 diagonal + E mask); batched tail; PE-transposed output.
Host unpermutes the output.
"""
import sys, os
sys.path.insert(0, '/opt/trn_rl_repo')

import numpy as np
import ml_dtypes
from contextlib import ExitStack

import concourse.bass as bass
import concourse.bacc as bacc
import concourse.tile as tile
from concourse import mybir
from concourse.bass_utils import run_bass_kernel_spmd

F32 = mybir.dt.float32
BF16 = mybir.dt.bfloat16
U16 = mybir.dt.uint16
I32 = mybir.dt.int32
AX = mybir.AxisListType
OP = mybir.AluOpType
ACTF = mybir.ActivationFunctionType

B, N, S = 4, 65536, 512
N_CORES = 8
NQ = N // 2
TQ = 128
NT = NQ // TQ
GRP = 16
EPS = 1e-8
KR = 21
SUB = 16
PAD = 32

_cache = {}
TRACE = False


def build_nc(slot_cands, goff, gspan_max):
    """slot_cands: per-slot candidate width; goff[g] = column offset of group g
    in the packed tables; gspan_max = max group span (pool sizing)."""
    nc = bacc.Bacc("TRN2", target_bir_lowering=False, debug=False,
                   num_devices=N_CORES)
    tot = int(goff[-1])
    lhs_d = nc.dram_tensor("lhs_d", [KR, NQ], BF16, kind="ExternalInput").ap()
    rhs_d = nc.dram_tensor("rhs_d", [KR, tot], BF16, kind="ExternalInput").ap()
    f2_d = nc.dram_tensor("f2_d", [1, tot], F32, kind="ExternalInput").ap()
    sq1_d = nc.dram_tensor("sq1_d", [128, NT], F32, kind="ExternalInput").ap()
    outc = nc.dram_tensor("outc", [NQ], F32, kind="ExternalOutput").ap()
    out2d = outc.rearrange("(t p) -> t p", p=TQ)

    with tile.TileContext(nc) as tc, ExitStack() as ctx:
        const = ctx.enter_context(tc.tile_pool(name="const", bufs=1))
        setup = ctx.enter_context(tc.tile_pool(name="setup", bufs=1))
        lt_pool = ctx.enter_context(tc.tile_pool(name="lt", bufs=3))
        rt_pool = ctx.enter_context(tc.tile_pool(name="rt", bufs=3))
        f2_pool = ctx.enter_context(tc.tile_pool(name="f2p", bufs=3))
        ps_P = ctx.enter_context(tc.tile_pool(name="psP", bufs=6, space="PSUM"))
        ps_T = ctx.enter_context(tc.tile_pool(name="psT", bufs=1, space="PSUM"))
        grp_pool = ctx.enter_context(tc.tile_pool(name="grp", bufs=6))
        tail_pool = ctx.enter_context(tc.tile_pool(name="tail", bufs=2))
        junk_pool = ctx.enter_context(tc.tile_pool(name="junk", bufs=2))
        stage_pool = ctx.enter_context(tc.tile_pool(name="stage", bufs=2))

        # E[p, i] = 1.0 iff (i % 16) == (p % 16)
        ramp = const.tile([128, 48], I32)
        nc.gpsimd.iota(ramp[:], pattern=[[0, 3], [1, 16]], base=0,
                       channel_multiplier=0)
        pid = const.tile([128, 48], I32)
        nc.gpsimd.iota(pid[:], pattern=[[0, 48]], base=0, channel_multiplier=1)
        pmod = const.tile([128, 48], I32)
        nc.vector.tensor_scalar(pmod[:], pid[:], 15, None, op0=OP.bitwise_and)
        E = const.tile([128, 48], F32)
        nc.vector.tensor_tensor(E[:], ramp[:], pmod[:], op=OP.is_equal)

        iot_p = const.tile([128, 128], I32)
        nc.gpsimd.iota(iot_p[:], pattern=[[0, 128]], base=0, channel_multiplier=1)
        iot_f = const.tile([128, 128], I32)
        nc.gpsimd.iota(iot_f[:], pattern=[[1, 128]], base=0, channel_multiplier=0)
        ident = const.tile([128, 128], F32)
        nc.vector.tensor_tensor(ident[:], iot_p[:], iot_f[:], op=OP.is_equal)

        sq1_sb = setup.tile([128, NT], F32)
        nc.sync.dma_start(sq1_sb[:], sq1_d[:])

        coff = [0]
        for t in range(NT):
            coff.append(coff[-1] + slot_cands[t])

        n_grp = NT // GRP
        stage = None
        for g in range(n_grp):
            stt_mode = g >= n_grp - 7
            m8g = grp_pool.tile([128, 8 * GRP], F32, tag="m8g")
            if not stt_mode:
                mig = grp_pool.tile([128, 8 * GRP], U16, tag="mig")
                g48g = grp_pool.tile([128, 48 * GRP], F32, tag="g48g")
            else:
                fg3 = grp_pool.tile([128, 3 * GRP], F32, tag="fg3")
            if g % (128 // GRP) == 0:
                stage = stage_pool.tile([128, 128], F32, tag="stage")

            o0, o1 = int(goff[g]), int(goff[g + 1])
            gspan = o1 - o0
            lt = lt_pool.tile([KR, TQ * GRP], BF16)
            nc.sync.dma_start(lt[:], lhs_d[:, g * GRP * TQ:(g + 1) * GRP * TQ])
            rtg = rt_pool.tile([KR, gspan_max], BF16, tag="rtg")
            nc.sync.dma_start(rtg[:, 0:gspan], rhs_d[:, o0:o1])
            f2g = f2_pool.tile([128, gspan_max], F32, tag="f2g")
            ck = (gspan + 3) // 4
            for q in range(4):
                a, bb = q * ck, min((q + 1) * ck, gspan)
                if a < bb:
                    nc.sync.dma_start(f2g[:, a:bb],
                                      f2_d[0:1, o0 + a:o0 + bb]
                                      .partition_broadcast(128))

            for j in range(GRP):
                t = g * GRP + j
                cnd = slot_cands[t]
                lo = coff[t] - o0
                pP = ps_P.tile([TQ, 512], F32, tag="pP")
                nc.tensor.matmul(pP[:, 0:cnd], lt[:, j * TQ:(j + 1) * TQ],
                                 rtg[:, lo:lo + cnd], start=True, stop=True)
                nc.vector.max(m8g[:, 8 * j:8 * j + 8], pP[:, 0:cnd])
                if stt_mode:
                    junk = junk_pool.tile([128, 512], F32, tag="junk")
                    for k in range(3):
                        nc.vector.scalar_tensor_tensor(
                            junk[:, 0:cnd], pP[:, 0:cnd],
                            m8g[:, 8 * j + k:8 * j + k + 1],
                            f2g[:, lo:lo + cnd],
                            op0=OP.is_equal, op1=OP.mult,
                            accum_out=fg3[:, 3 * j + k:3 * j + k + 1])
                    continue
                nc.vector.max_index(mig[:, 8 * j:8 * j + 8],
                                    m8g[:, 8 * j:8 * j + 8], pP[:, 0:cnd])
                if j > 0:
                    jp = j - 1
                    tp = g * GRP + jp
                    lop = coff[tp] - o0
                    nc.gpsimd.indirect_copy(g48g[:, 48 * jp:48 * jp + 48],
                                            f2g[:, lop:lop + slot_cands[tp]],
                                            mig[:, 8 * jp:8 * jp + 3],
                                            i_know_ap_gather_is_preferred=True)
            if not stt_mode:
                jp = GRP - 1
                tp = g * GRP + jp
                lop = coff[tp] - o0
                nc.gpsimd.indirect_copy(g48g[:, 48 * jp:48 * jp + 48],
                                        f2g[:, lop:lop + slot_cands[tp]],
                                        mig[:, 8 * jp:8 * jp + 3],
                                        i_know_ap_gather_is_preferred=True)

            # batched tail
            m3 = m8g[:].rearrange("p (j e) -> p j e", e=8)[:, :, 0:3]
            sq1r = sq1_sb[:, g * GRP:(g + 1) * GRP].unsqueeze(-1) \
                                                   .broadcast_to([128, GRP, 3])
            d3 = tail_pool.tile([128, 3 * GRP], F32, tag="d3")
            d3v = d3[:].rearrange("p (j e) -> p j e", e=3)
            nc.vector.tensor_tensor(d3v, sq1r, m3, op=OP.subtract)
            r = tail_pool.tile([128, 3 * GRP], F32, tag="r")
            nc.vector.reciprocal(r[:], d3[:])
            den = tail_pool.tile([128, GRP], F32, tag="den")
            nc.vector.reduce_sum(den[:], r[:].rearrange("p (j e) -> p j e", e=3),
                                 axis=AX.X)
            num = tail_pool.tile([128, GRP], F32, tag="num")
            if stt_mode:
                t1s = tail_pool.tile([128, 3 * GRP], F32, tag="t1s")
                nc.vector.tensor_tensor(t1s[:], r[:], fg3[:], op=OP.mult)
                nc.vector.reduce_sum(num[:],
                                     t1s[:].rearrange("p (j e) -> p j e", e=3),
                                     axis=AX.X)
            else:
                r_rep = r[:].rearrange("p (j e) -> p j e", e=3).unsqueeze(-1) \
                            .broadcast_to([128, GRP, 3, 16])
                g4 = g48g[:].rearrange("p (j k q) -> p j k q", k=3, q=16)
                t1 = tail_pool.tile([128, 48 * GRP], F32, tag="t1")
                t1v = t1[:].rearrange("p (j k q) -> p j k q", k=3, q=16)
                nc.vector.tensor_tensor(t1v, g4, r_rep, op=OP.mult)
                e_rep = E[:].unsqueeze(1).broadcast_to([128, GRP, 48])
                t2 = tail_pool.tile([128, 48 * GRP], F32, tag="t2")
                t2v = t2[:].rearrange("p (j i) -> p j i", i=48)
                nc.vector.tensor_tensor(t2v,
                                        t1[:].rearrange("p (j i) -> p j i", i=48),
                                        e_rep, op=OP.mult)
                nc.vector.reduce_sum(num[:], t2v, axis=AX.X)
            rden = tail_pool.tile([128, GRP], F32, tag="rden")
            nc.vector.reciprocal(rden[:], den[:])
            outv = tail_pool.tile([128, GRP], F32, tag="outv")
            nc.vector.tensor_tensor(outv[:], num[:], rden[:], op=OP.mult)
            col = (g * GRP) % 128
            nc.scalar.activation(stage[:, col:col + GRP], outv[:], ACTF.Sigmoid,
                                 scale=2.0)

            if (g + 1) % (128 // GRP) == 0:
                blk = (g * GRP) // 128
                pT = ps_T.tile([128, 128], F32)
                nc.tensor.transpose(pT[:], stage[:], ident[:])
                oT = stage_pool.tile([128, 128], F32, tag="oT")
                nc.scalar.copy(oT[:], pT[:])
                nc.sync.dma_start(out2d[blk * 128:(blk + 1) * 128, :], oT[:])

    nc.compile()
    return nc


def _split3(v32):
    h = v32.astype(ml_dtypes.bfloat16)
    rr = (v32 - h.astype(np.float32)).astype(np.float32)
    m = rr.astype(ml_dtypes.bfloat16)
    l = (rr - m.astype(np.float32)).astype(ml_dtypes.bfloat16)
    return h, m, l


def _kd_order(pts, leaf):
    idx = np.arange(len(pts))
    out = []
    stack = [idx]
    while stack:
        ids = stack.pop()
        if len(ids) <= leaf:
            out.append(ids)
            continue
        p = pts[ids]
        ax = int(np.argmax(p.max(0) - p.min(0)))
        k = (len(ids) // 2 // leaf) * leaf
        if k == 0:
            k = len(ids) // 2
        part = np.argpartition(p[:, ax], k)
        stack.append(ids[part[k:]])
        stack.append(ids[part[:k]])
    return np.concatenate(out)


def _prep_core(xyz1h, xyz2b, f2row):
    """Returns dict with order, per-tile candidate index lists, counts."""
    q = xyz1h.T.astype(np.float64)          # [NQ, 3]
    r = xyz2b.T.astype(np.float64)          # [S, 3]
    order = _kd_order(q, SUB)
    qs = q[order]
    nsub = NQ // SUB
    qsub = qs.reshape(nsub, SUB, 3)
    c = qsub.mean(1)
    rho = np.sqrt(((qsub - c[:, None, :]) ** 2).sum(2)).max(1)
    dc = np.sqrt(((c[:, None, :] - r[None, :, :]) ** 2).sum(2))
    d3c = np.partition(dc, 2, axis=1)[:, 2]
    need = dc <= (d3c + 2 * rho + 1e-3)[:, None]
    need_t = need.reshape(NT, TQ // SUB, S).any(1)      # [NT, S]
    counts = need_t.sum(1)
    padded = np.maximum(PAD, ((counts + PAD - 1) // PAD) * PAD)
    tile_rank = np.argsort(-padded, kind='stable')       # slot -> original tile
    return {"order": order, "need_t": need_t, "padded": padded,
            "tile_rank": tile_rank}


def _make_core_inputs(xyz1h, xyz2b, f2row, prep, slot_cands, goff):
    order = prep["order"]
    need_t = prep["need_t"]
    tile_rank = prep["tile_rank"]

    x1 = xyz1h.astype(np.float32)[:, order]              # permuted queries
    x2 = xyz2b.astype(np.float32)
    f2 = f2row.astype(np.float32).reshape(-1)

    xh, xm, xl = {}, {}, {}
    for cc in range(3):
        xh[cc], xm[cc], xl[cc] = _split3(x1[cc])
    yh, ym, yl = {}, {}, {}
    for cc in range(3):
        yh[cc], ym[cc], yl[cc] = _split3((2.0 * x2[cc]).astype(np.float32))
    sq2 = ((x2[0] * x2[0] + x2[1] * x2[1]) + x2[2] * x2[2]).astype(np.float32)
    sh, sm, sl = _split3(-sq2)

    onesq = np.ones(NQ, ml_dtypes.bfloat16)
    lhs_rows, rhs_rows = [], []
    for cc in range(3):
        lhs_rows.append(xh[cc]); rhs_rows.append(yh[cc])
    lhs_rows.append(onesq); rhs_rows.append(sh)
    for cc in range(3):
        lhs_rows.append(xh[cc]); rhs_rows.append(ym[cc])
        lhs_rows.append(xm[cc]); rhs_rows.append(yh[cc])
    lhs_rows.append(onesq); rhs_rows.append(sm)
    for cc in range(3):
        lhs_rows.append(xh[cc]); rhs_rows.append(yl[cc])
        lhs_rows.append(xl[cc]); rhs_rows.append(yh[cc])
        lhs_rows.append(xm[cc]); rhs_rows.append(ym[cc])
    lhs_rows.append(onesq); rhs_rows.append(sl)
    lhs = np.stack(lhs_rows).astype(ml_dtypes.bfloat16)      # [KR, NQ]
    rhs_full = np.stack([np.asarray(rr_, np.float32) for rr_ in rhs_rows]) \
                 .astype(np.float32)                          # [KR, S] fp32 view
    rhs_full_bf = np.stack(rhs_rows).astype(ml_dtypes.bfloat16)

    tot = int(goff[-1])
    rhs_pack = np.zeros((KR, tot), ml_dtypes.bfloat16)
    f2_pack = np.zeros((1, tot), np.float32)
    SQ2H_ROW = 3   # the sh row index
    off = 0
    for slot in range(NT):
        t_orig = tile_rank[slot]
        cidx = np.nonzero(need_t[t_orig])[0]
        w = int(slot_cands[slot])
        assert len(cidx) <= w, (slot, len(cidx), w)
        rhs_pack[:, off:off + len(cidx)] = rhs_full_bf[:, cidx]
        if len(cidx) < w:
            rhs_pack[SQ2H_ROW, off + len(cidx):off + w] = \
                ml_dtypes.bfloat16(-1e30)
        f2_pack[0, off:off + len(cidx)] = f2[cidx]
        off += w
    assert off == tot

    # lhs permuted additionally by tile rank: slot s covers original tile
    # tile_rank[s], i.e. queries order[tile_rank[s]*128 : +128]
    qsel = np.concatenate([np.arange(tile_rank[s] * TQ, tile_rank[s] * TQ + TQ)
                           for s in range(NT)])
    lhs_slot = np.ascontiguousarray(lhs[:, qsel])

    sq1 = ((x1[0] * x1[0] + x1[1] * x1[1]) + x1[2] * x1[2]).astype(np.float32)
    sq1e = (sq1 + np.float32(EPS)).astype(np.float32)[qsel]
    sq1_t = np.ascontiguousarray(sq1e.reshape(NT, TQ).T)

    # final query permutation: device position i corresponds to
    # original query order[qsel[i]]
    perm = order[qsel]

    return {
        "lhs_d": lhs_slot,
        "rhs_d": np.ascontiguousarray(rhs_pack),
        "f2_d": np.ascontiguousarray(f2_pack),
        "sq1_d": sq1_t,
    }, perm


def kernel(xyz1, xyz2, points2):
    xyz1 = np.ascontiguousarray(np.asarray(xyz1, dtype=np.float32))
    xyz2 = np.ascontiguousarray(np.asarray(xyz2, dtype=np.float32))
    points2 = np.ascontiguousarray(np.asarray(points2, dtype=np.float32))

    preps = []
    for c in range(N_CORES):
        b, h = c // 2, c % 2
        preps.append(_prep_core(xyz1[b][:, h * NQ:(h + 1) * NQ],
                                xyz2[b], points2[b]))
    padded_sorted = np.stack([np.sort(p["padded"])[::-1] for p in preps])
    slot_cands = padded_sorted.max(0).astype(np.int64)       # [NT]
    goff = np.zeros(NT // GRP + 1, np.int64)
    for g in range(NT // GRP):
        goff[g + 1] = goff[g] + slot_cands[g * GRP:(g + 1) * GRP].sum()
    gspan_max = int(max(goff[g + 1] - goff[g] for g in range(NT // GRP)))

    key = tuple(slot_cands.tolist())
    if key not in _cache:
        _cache[key] = build_nc([int(x) for x in slot_cands], goff, gspan_max)
    nc = _cache[key]

    in_maps, perms = [], []
    for c in range(N_CORES):
        b, h = c // 2, c % 2
        im, perm = _make_core_inputs(xyz1[b][:, h * NQ:(h + 1) * NQ],
                                     xyz2[b], points2[b],
                                     preps[c], slot_cands, goff)
        in_maps.append(im)
        perms.append(perm)

    res = run_bass_kernel_spmd(nc, in_maps, core_ids=list(range(N_CORES)),
                               trace=TRACE)
    if TRACE:
        _cache["last_exec_time_ns"] = res.exec_time_ns
    out = np.empty((B, N), dtype=np.float32)
    for c in range(N_CORES):
        b, h = c // 2, c % 2
        seg = np.empty(NQ, np.float32)
        seg[perms[c]] = res.results[c]["outc"]
        out[b, h * NQ:(h + 1) * NQ] = seg
    return out


if __name__ == "__main__":
    rng = np.random.default_rng(0)
    xyz1 = rng.standard_normal((B, 3, N)).astype(np.float32)
    xyz2 = rng.standard_normal((B, 3, S)).astype(np.float32)
    points2 = rng.standard_normal((B, 1, S)).astype(np.float32)
    out = kernel(xyz1, xyz2, points2)
    print(out.shape, out[0, :5])


# revision 32
# speedup vs baseline: 1.3901x; 1.1092x over previous
"""Trainium2 Bass kernel for 3-NN IDW interpolation — spatially pruned version.

Host prep (cheap numpy): per core, queries are kd-ordered into 256 tiles of
128 spatially-coherent queries. Each tile gets a provably sufficient
candidate set (union of sub-cluster bounds: any reference that could be a
top-3 neighbor of any query in the tile). Tiles are processed sorted by
candidate count descending; the SPMD program's per-slot candidate width is
the max across the 8 cores at that slot rank.

Device per tile: bf16-split matmul P = 2*dot - sq2 over candidate columns;
DVE max8 + max_index; GPSIMD shared-column gather of the tile-local f2
table (mod-16 diagonal + E mask); batched tail; PE-transposed output.
Host unpermutes the output.
"""
import sys, os
sys.path.insert(0, '/opt/trn_rl_repo')

import numpy as np
import ml_dtypes
from contextlib import ExitStack

import concourse.bass as bass
import concourse.bacc as bacc
import concourse.tile as tile
from concourse import mybir
from concourse.bass_utils import run_bass_kernel_spmd

F32 = mybir.dt.float32
BF16 = mybir.dt.bfloat16
U16 = mybir.dt.uint16
I32 = mybir.dt.int32
AX = mybir.AxisListType
OP = mybir.AluOpType
ACTF = mybir.ActivationFunctionType

B, N, S = 4, 65536, 512
N_CORES = 8
NQ = N // 2
TQ = 128
NT = NQ // TQ
GRP = 16
EPS = 1e-8
KR = 21
SUB = 16
PAD = 32

_cache = {}
TRACE = False


def build_nc(slot_cands, goff, gspan_max):
    """slot_cands: per-slot candidate width; goff[g] = column offset of group g
    in the packed tables; gspan_max = max group span (pool sizing)."""
    nc = bacc.Bacc("TRN2", target_bir_lowering=False, debug=False,
                   num_devices=N_CORES)
    tot = int(goff[-1])
    lhs_d = nc.dram_tensor("lhs_d", [KR, NQ], BF16, kind="ExternalInput").ap()
    rhs_d = nc.dram_tensor("rhs_d", [KR, tot], BF16, kind="ExternalInput").ap()
    f2_d = nc.dram_tensor("f2_d", [1, tot], F32, kind="ExternalInput").ap()
    sq1_d = nc.dram_tensor("sq1_d", [128, NT], F32, kind="ExternalInput").ap()
    outc = nc.dram_tensor("outc", [NQ], F32, kind="ExternalOutput").ap()
    out2d = outc.rearrange("(t p) -> t p", p=TQ)

    with tile.TileContext(nc) as tc, ExitStack() as ctx:
        const = ctx.enter_context(tc.tile_pool(name="const", bufs=1))
        setup = ctx.enter_context(tc.tile_pool(name="setup", bufs=1))
        lt_pool = ctx.enter_context(tc.tile_pool(name="lt", bufs=3))
        rt_pool = ctx.enter_context(tc.tile_pool(name="rt", bufs=3))
        f2_pool = ctx.enter_context(tc.tile_pool(name="f2p", bufs=3))
        ps_P = ctx.enter_context(tc.tile_pool(name="psP", bufs=6, space="PSUM"))
        ps_T = ctx.enter_context(tc.tile_pool(name="psT", bufs=1, space="PSUM"))
        grp_pool = ctx.enter_context(tc.tile_pool(name="grp", bufs=6))
        tail_pool = ctx.enter_context(tc.tile_pool(name="tail", bufs=2))
        junk_pool = ctx.enter_context(tc.tile_pool(name="junk", bufs=2))
        stage_pool = ctx.enter_context(tc.tile_pool(name="stage", bufs=2))

        # E[p, i] = 1.0 iff (i % 16) == (p % 16)
        ramp = const.tile([128, 48], I32)
        nc.gpsimd.iota(ramp[:], pattern=[[0, 3], [1, 16]], base=0,
                       channel_multiplier=0)
        pid = const.tile([128, 48], I32)
        nc.gpsimd.iota(pid[:], pattern=[[0, 48]], base=0, channel_multiplier=1)
        pmod = const.tile([128, 48], I32)
        nc.vector.tensor_scalar(pmod[:], pid[:], 15, None, op0=OP.bitwise_and)
        E = const.tile([128, 48], F32)
        nc.vector.tensor_tensor(E[:], ramp[:], pmod[:], op=OP.is_equal)

        iot_p = const.tile([128, 128], I32)
        nc.gpsimd.iota(iot_p[:], pattern=[[0, 128]], base=0, channel_multiplier=1)
        iot_f = const.tile([128, 128], I32)
        nc.gpsimd.iota(iot_f[:], pattern=[[1, 128]], base=0, channel_multiplier=0)
        ident = const.tile([128, 128], F32)
        nc.vector.tensor_tensor(ident[:], iot_p[:], iot_f[:], op=OP.is_equal)

        sq1_sb = setup.tile([128, NT], F32)
        nc.sync.dma_start(sq1_sb[:], sq1_d[:])

        coff = [0]
        for t in range(NT):
            coff.append(coff[-1] + slot_cands[t])

        n_grp = NT // GRP
        stage = None
        for g in range(n_grp):
            stt_mode = g >= n_grp - 10
            m8g = grp_pool.tile([128, 8 * GRP], F32, tag="m8g")
            if not stt_mode:
                mig = grp_pool.tile([128, 8 * GRP], U16, tag="mig")
                g48g = grp_pool.tile([128, 48 * GRP], F32, tag="g48g")
            else:
                fg3 = grp_pool.tile([128, 3 * GRP], F32, tag="fg3")
            if g % (128 // GRP) == 0:
                stage = stage_pool.tile([128, 128], F32, tag="stage")

            o0, o1 = int(goff[g]), int(goff[g + 1])
            gspan = o1 - o0
            lt = lt_pool.tile([KR, TQ * GRP], BF16)
            nc.sync.dma_start(lt[:], lhs_d[:, g * GRP * TQ:(g + 1) * GRP * TQ])
            rtg = rt_pool.tile([KR, gspan_max], BF16, tag="rtg")
            nc.sync.dma_start(rtg[:, 0:gspan], rhs_d[:, o0:o1])
            f2g = f2_pool.tile([128, gspan_max], F32, tag="f2g")
            ck = (gspan + 3) // 4
            for q in range(4):
                a, bb = q * ck, min((q + 1) * ck, gspan)
                if a < bb:
                    nc.sync.dma_start(f2g[:, a:bb],
                                      f2_d[0:1, o0 + a:o0 + bb]
                                      .partition_broadcast(128))

            for j in range(GRP):
                t = g * GRP + j
                cnd = slot_cands[t]
                lo = coff[t] - o0
                pP = ps_P.tile([TQ, 512], F32, tag="pP")
                nc.tensor.matmul(pP[:, 0:cnd], lt[:, j * TQ:(j + 1) * TQ],
                                 rtg[:, lo:lo + cnd], start=True, stop=True)
                nc.vector.max(m8g[:, 8 * j:8 * j + 8], pP[:, 0:cnd])
                if stt_mode:
                    junk = junk_pool.tile([128, 512], F32, tag="junk")
                    for k in range(3):
                        nc.vector.scalar_tensor_tensor(
                            junk[:, 0:cnd], pP[:, 0:cnd],
                            m8g[:, 8 * j + k:8 * j + k + 1],
                            f2g[:, lo:lo + cnd],
                            op0=OP.is_equal, op1=OP.mult,
                            accum_out=fg3[:, 3 * j + k:3 * j + k + 1])
                    continue
                nc.vector.max_index(mig[:, 8 * j:8 * j + 8],
                                    m8g[:, 8 * j:8 * j + 8], pP[:, 0:cnd])
                if j > 0:
                    jp = j - 1
                    tp = g * GRP + jp
                    lop = coff[tp] - o0
                    nc.gpsimd.indirect_copy(g48g[:, 48 * jp:48 * jp + 48],
                                            f2g[:, lop:lop + slot_cands[tp]],
                                            mig[:, 8 * jp:8 * jp + 3],
                                            i_know_ap_gather_is_preferred=True)
            if not stt_mode:
                jp = GRP - 1
                tp = g * GRP + jp
                lop = coff[tp] - o0
                nc.gpsimd.indirect_copy(g48g[:, 48 * jp:48 * jp + 48],
                                        f2g[:, lop:lop + slot_cands[tp]],
                                        mig[:, 8 * jp:8 * jp + 3],
                                        i_know_ap_gather_is_preferred=True)

            # batched tail
            m3 = m8g[:].rearrange("p (j e) -> p j e", e=8)[:, :, 0:3]
            sq1r = sq1_sb[:, g * GRP:(g + 1) * GRP].unsqueeze(-1) \
                                                   .broadcast_to([128, GRP, 3])
            d3 = tail_pool.tile([128, 3 * GRP], F32, tag="d3")
            d3v = d3[:].rearrange("p (j e) -> p j e", e=3)
            nc.vector.tensor_tensor(d3v, sq1r, m3, op=OP.subtract)
            r = tail_pool.tile([128, 3 * GRP], F32, tag="r")
            nc.vector.reciprocal(r[:], d3[:])
            den = tail_pool.tile([128, GRP], F32, tag="den")
            nc.vector.reduce_sum(den[:], r[:].rearrange("p (j e) -> p j e", e=3),
                                 axis=AX.X)
            num = tail_pool.tile([128, GRP], F32, tag="num")
            if stt_mode:
                t1s = tail_pool.tile([128, 3 * GRP], F32, tag="t1s")
                nc.vector.tensor_tensor(t1s[:], r[:], fg3[:], op=OP.mult)
                nc.vector.reduce_sum(num[:],
                                     t1s[:].rearrange("p (j e) -> p j e", e=3),
                                     axis=AX.X)
            else:
                r_rep = r[:].rearrange("p (j e) -> p j e", e=3).unsqueeze(-1) \
                            .broadcast_to([128, GRP, 3, 16])
                g4 = g48g[:].rearrange("p (j k q) -> p j k q", k=3, q=16)
                t1 = tail_pool.tile([128, 48 * GRP], F32, tag="t1")
                t1v = t1[:].rearrange("p (j k q) -> p j k q", k=3, q=16)
                nc.vector.tensor_tensor(t1v, g4, r_rep, op=OP.mult)
                e_rep = E[:].unsqueeze(1).broadcast_to([128, GRP, 48])
                t2 = tail_pool.tile([128, 48 * GRP], F32, tag="t2")
                t2v = t2[:].rearrange("p (j i) -> p j i", i=48)
                nc.vector.tensor_tensor(t2v,
                                        t1[:].rearrange("p (j i) -> p j i", i=48),
                                        e_rep, op=OP.mult)
                nc.vector.reduce_sum(num[:], t2v, axis=AX.X)
            rden = tail_pool.tile([128, GRP], F32, tag="rden")
            nc.vector.reciprocal(rden[:], den[:])
            outv = tail_pool.tile([128, GRP], F32, tag="outv")
            nc.vector.tensor_tensor(outv[:], num[:], rden[:], op=OP.mult)
            col = (g * GRP) % 128
            nc.scalar.activation(stage[:, col:col + GRP], outv[:], ACTF.Sigmoid,
                                 scale=2.0)

            if (g + 1) % (128 // GRP) == 0:
                blk = (g * GRP) // 128
                pT = ps_T.tile([128, 128], F32)
                nc.tensor.transpose(pT[:], stage[:], ident[:])
                oT = stage_pool.tile([128, 128], F32, tag="oT")
                nc.scalar.copy(oT[:], pT[:])
                nc.sync.dma_start(out2d[blk * 128:(blk + 1) * 128, :], oT[:])

    nc.compile()
    return nc


def _split3(v32):
    h = v32.astype(ml_dtypes.bfloat16)
    rr = (v32 - h.astype(np.float32)).astype(np.float32)
    m = rr.astype(ml_dtypes.bfloat16)
    l = (rr - m.astype(np.float32)).astype(ml_dtypes.bfloat16)
    return h, m, l


def _kd_order(pts, leaf):
    idx = np.arange(len(pts))
    out = []
    stack = [idx]
    while stack:
        ids = stack.pop()
        if len(ids) <= leaf:
            out.append(ids)
            continue
        p = pts[ids]
        ax = int(np.argmax(p.max(0) - p.min(0)))
        k = (len(ids) // 2 // leaf) * leaf
        if k == 0:
            k = len(ids) // 2
        part = np.argpartition(p[:, ax], k)
        stack.append(ids[part[k:]])
        stack.append(ids[part[:k]])
    return np.concatenate(out)


def _prep_core(xyz1h, xyz2b, f2row):
    """Returns dict with order, per-tile candidate index lists, counts."""
    q = xyz1h.T.astype(np.float64)          # [NQ, 3]
    r = xyz2b.T.astype(np.float64)          # [S, 3]
    order = _kd_order(q, SUB)
    qs = q[order]
    nsub = NQ // SUB
    qsub = qs.reshape(nsub, SUB, 3)
    c = qsub.mean(1)
    rho = np.sqrt(((qsub - c[:, None, :]) ** 2).sum(2)).max(1)
    dc = np.sqrt(((c[:, None, :] - r[None, :, :]) ** 2).sum(2))
    d3c = np.partition(dc, 2, axis=1)[:, 2]
    need = dc <= (d3c + 2 * rho + 1e-3)[:, None]
    need_t = need.reshape(NT, TQ // SUB, S).any(1)      # [NT, S]
    counts = need_t.sum(1)
    padded = np.maximum(PAD, ((counts + PAD - 1) // PAD) * PAD)
    tile_rank = np.argsort(-padded, kind='stable')       # slot -> original tile
    return {"order": order, "need_t": need_t, "padded": padded,
            "tile_rank": tile_rank}


def _make_core_inputs(xyz1h, xyz2b, f2row, prep, slot_cands, goff):
    order = prep["order"]
    need_t = prep["need_t"]
    tile_rank = prep["tile_rank"]

    x1 = xyz1h.astype(np.float32)[:, order]              # permuted queries
    x2 = xyz2b.astype(np.float32)
    f2 = f2row.astype(np.float32).reshape(-1)

    xh, xm, xl = {}, {}, {}
    for cc in range(3):
        xh[cc], xm[cc], xl[cc] = _split3(x1[cc])
    yh, ym, yl = {}, {}, {}
    for cc in range(3):
        yh[cc], ym[cc], yl[cc] = _split3((2.0 * x2[cc]).astype(np.float32))
    sq2 = ((x2[0] * x2[0] + x2[1] * x2[1]) + x2[2] * x2[2]).astype(np.float32)
    sh, sm, sl = _split3(-sq2)

    onesq = np.ones(NQ, ml_dtypes.bfloat16)
    lhs_rows, rhs_rows = [], []
    for cc in range(3):
        lhs_rows.append(xh[cc]); rhs_rows.append(yh[cc])
    lhs_rows.append(onesq); rhs_rows.append(sh)
    for cc in range(3):
        lhs_rows.append(xh[cc]); rhs_rows.append(ym[cc])
        lhs_rows.append(xm[cc]); rhs_rows.append(yh[cc])
    lhs_rows.append(onesq); rhs_rows.append(sm)
    for cc in range(3):
        lhs_rows.append(xh[cc]); rhs_rows.append(yl[cc])
        lhs_rows.append(xl[cc]); rhs_rows.append(yh[cc])
        lhs_rows.append(xm[cc]); rhs_rows.append(ym[cc])
    lhs_rows.append(onesq); rhs_rows.append(sl)
    lhs = np.stack(lhs_rows).astype(ml_dtypes.bfloat16)      # [KR, NQ]
    rhs_full = np.stack([np.asarray(rr_, np.float32) for rr_ in rhs_rows]) \
                 .astype(np.float32)                          # [KR, S] fp32 view
    rhs_full_bf = np.stack(rhs_rows).astype(ml_dtypes.bfloat16)

    tot = int(goff[-1])
    rhs_pack = np.zeros((KR, tot), ml_dtypes.bfloat16)
    f2_pack = np.zeros((1, tot), np.float32)
    SQ2H_ROW = 3   # the sh row index
    off = 0
    for slot in range(NT):
        t_orig = tile_rank[slot]
        cidx = np.nonzero(need_t[t_orig])[0]
        w = int(slot_cands[slot])
        assert len(cidx) <= w, (slot, len(cidx), w)
        rhs_pack[:, off:off + len(cidx)] = rhs_full_bf[:, cidx]
        if len(cidx) < w:
            rhs_pack[SQ2H_ROW, off + len(cidx):off + w] = \
                ml_dtypes.bfloat16(-1e30)
        f2_pack[0, off:off + len(cidx)] = f2[cidx]
        off += w
    assert off == tot

    # lhs permuted additionally by tile rank: slot s covers original tile
    # tile_rank[s], i.e. queries order[tile_rank[s]*128 : +128]
    qsel = np.concatenate([np.arange(tile_rank[s] * TQ, tile_rank[s] * TQ + TQ)
                           for s in range(NT)])
    lhs_slot = np.ascontiguousarray(lhs[:, qsel])

    sq1 = ((x1[0] * x1[0] + x1[1] * x1[1]) + x1[2] * x1[2]).astype(np.float32)
    sq1e = (sq1 + np.float32(EPS)).astype(np.float32)[qsel]
    sq1_t = np.ascontiguousarray(sq1e.reshape(NT, TQ).T)

    # final query permutation: device position i corresponds to
    # original query order[qsel[i]]
    perm = order[qsel]

    return {
        "lhs_d": lhs_slot,
        "rhs_d": np.ascontiguousarray(rhs_pack),
        "f2_d": np.ascontiguousarray(f2_pack),
        "sq1_d": sq1_t,
    }, perm


def kernel(xyz1, xyz2, points2):
    xyz1 = np.ascontiguousarray(np.asarray(xyz1, dtype=np.float32))
    xyz2 = np.ascontiguousarray(np.asarray(xyz2, dtype=np.float32))
    points2 = np.ascontiguousarray(np.asarray(points2, dtype=np.float32))

    preps = []
    for c in range(N_CORES):
        b, h = c // 2, c % 2
        preps.append(_prep_core(xyz1[b][:, h * NQ:(h + 1) * NQ],
                                xyz2[b], points2[b]))
    padded_sorted = np.stack([np.sort(p["padded"])[::-1] for p in preps])
    slot_cands = padded_sorted.max(0).astype(np.int64)       # [NT]
    goff = np.zeros(NT // GRP + 1, np.int64)
    for g in range(NT // GRP):
        goff[g + 1] = goff[g] + slot_cands[g * GRP:(g + 1) * GRP].sum()
    gspan_max = int(max(goff[g + 1] - goff[g] for g in range(NT // GRP)))

    key = tuple(slot_cands.tolist())
    if key not in _cache:
        _cache[key] = build_nc([int(x) for x in slot_cands], goff, gspan_max)
    nc = _cache[key]

    in_maps, perms = [], []
    for c in range(N_CORES):
        b, h = c // 2, c % 2
        im, perm = _make_core_inputs(xyz1[b][:, h * NQ:(h + 1) * NQ],
                                     xyz2[b], points2[b],
                                     preps[c], slot_cands, goff)
        in_maps.append(im)
        perms.append(perm)

    res = run_bass_kernel_spmd(nc, in_maps, core_ids=list(range(N_CORES)),
                               trace=TRACE)
    if TRACE:
        _cache["last_exec_time_ns"] = res.exec_time_ns
    out = np.empty((B, N), dtype=np.float32)
    for c in range(N_CORES):
        b, h = c // 2, c % 2
        seg = np.empty(NQ, np.float32)
        seg[perms[c]] = res.results[c]["outc"]
        out[b, h * NQ:(h + 1) * NQ] = seg
    return out


if __name__ == "__main__":
    rng = np.random.default_rng(0)
    xyz1 = rng.standard_normal((B, 3, N)).astype(np.float32)
    xyz2 = rng.standard_normal((B, 3, S)).astype(np.float32)
    points2 = rng.standard_normal((B, 1, S)).astype(np.float32)
    out = kernel(xyz1, xyz2, points2)
    print(out.shape, out[0, :5])


# revision 33
# speedup vs baseline: 1.4834x; 1.0672x over previous
"""Trainium2 Bass kernel for 3-NN IDW interpolation — spatially pruned version.

Host prep (cheap numpy): per core, queries are kd-ordered into 256 tiles of
128 spatially-coherent queries. Each tile gets a provably sufficient
candidate set (union of sub-cluster bounds: any reference that could be a
top-3 neighbor of any query in the tile). Tiles are processed sorted by
candidate count descending; the SPMD program's per-slot candidate width is
the max across the 8 cores at that slot rank.

Device per tile: bf16-split matmul P = 2*dot - sq2 over candidate columns;
DVE max8 + max_index; GPSIMD shared-column gather of the tile-local f2
table (mod-16 diagonal + E mask); batched tail; PE-transposed output.
Host unpermutes the output.
"""
import sys, os
sys.path.insert(0, '/opt/trn_rl_repo')

import numpy as np
import ml_dtypes
from contextlib import ExitStack

import concourse.bass as bass
import concourse.bacc as bacc
import concourse.tile as tile
from concourse import mybir
from concourse.bass_utils import run_bass_kernel_spmd

F32 = mybir.dt.float32
BF16 = mybir.dt.bfloat16
U16 = mybir.dt.uint16
I32 = mybir.dt.int32
AX = mybir.AxisListType
OP = mybir.AluOpType
ACTF = mybir.ActivationFunctionType

B, N, S = 4, 65536, 512
N_CORES = 8
NQ = N // 2
TQ = 128
NT = NQ // TQ
GRP = 16
EPS = 1e-8
KR = 21
SUB = 16
PAD = 32

_cache = {}
TRACE = False


def build_nc(slot_cands, goff, gspan_max):
    """slot_cands: per-slot candidate width; goff[g] = column offset of group g
    in the packed tables; gspan_max = max group span (pool sizing)."""
    nc = bacc.Bacc("TRN2", target_bir_lowering=False, debug=False,
                   num_devices=N_CORES)
    tot = int(goff[-1])
    lhs_d = nc.dram_tensor("lhs_d", [KR, NQ], BF16, kind="ExternalInput").ap()
    rhs_d = nc.dram_tensor("rhs_d", [KR, tot], BF16, kind="ExternalInput").ap()
    f2_d = nc.dram_tensor("f2_d", [1, tot], F32, kind="ExternalInput").ap()
    sq1_d = nc.dram_tensor("sq1_d", [128, NT], F32, kind="ExternalInput").ap()
    outc = nc.dram_tensor("outc", [NQ], F32, kind="ExternalOutput").ap()
    out2d = outc.rearrange("(t p) -> t p", p=TQ)

    with tile.TileContext(nc) as tc, ExitStack() as ctx:
        const = ctx.enter_context(tc.tile_pool(name="const", bufs=1))
        setup = ctx.enter_context(tc.tile_pool(name="setup", bufs=1))
        lt_pool = ctx.enter_context(tc.tile_pool(name="lt", bufs=3))
        rt_pool = ctx.enter_context(tc.tile_pool(name="rt", bufs=3))
        f2_pool = ctx.enter_context(tc.tile_pool(name="f2p", bufs=3))
        ps_P = ctx.enter_context(tc.tile_pool(name="psP", bufs=6, space="PSUM"))
        ps_T = ctx.enter_context(tc.tile_pool(name="psT", bufs=1, space="PSUM"))
        grp_pool = ctx.enter_context(tc.tile_pool(name="grp", bufs=6))
        tail_pool = ctx.enter_context(tc.tile_pool(name="tail", bufs=2))
        junk_pool = ctx.enter_context(tc.tile_pool(name="junk", bufs=2))
        stage_pool = ctx.enter_context(tc.tile_pool(name="stage", bufs=2))

        # E[p, i] = 1.0 iff (i % 16) == (p % 16)
        ramp = const.tile([128, 48], I32)
        nc.gpsimd.iota(ramp[:], pattern=[[0, 3], [1, 16]], base=0,
                       channel_multiplier=0)
        pid = const.tile([128, 48], I32)
        nc.gpsimd.iota(pid[:], pattern=[[0, 48]], base=0, channel_multiplier=1)
        pmod = const.tile([128, 48], I32)
        nc.vector.tensor_scalar(pmod[:], pid[:], 15, None, op0=OP.bitwise_and)
        E = const.tile([128, 48], F32)
        nc.vector.tensor_tensor(E[:], ramp[:], pmod[:], op=OP.is_equal)

        iot_p = const.tile([128, 128], I32)
        nc.gpsimd.iota(iot_p[:], pattern=[[0, 128]], base=0, channel_multiplier=1)
        iot_f = const.tile([128, 128], I32)
        nc.gpsimd.iota(iot_f[:], pattern=[[1, 128]], base=0, channel_multiplier=0)
        ident = const.tile([128, 128], F32)
        nc.vector.tensor_tensor(ident[:], iot_p[:], iot_f[:], op=OP.is_equal)

        sq1_sb = setup.tile([128, NT], F32)
        nc.sync.dma_start(sq1_sb[:], sq1_d[:])

        coff = [0]
        for t in range(NT):
            coff.append(coff[-1] + slot_cands[t])

        n_grp = NT // GRP
        stage = None
        for g in range(n_grp):
            stt_mode = g >= n_grp - 13
            m8g = grp_pool.tile([128, 8 * GRP], F32, tag="m8g")
            if not stt_mode:
                mig = grp_pool.tile([128, 8 * GRP], U16, tag="mig")
                g48g = grp_pool.tile([128, 48 * GRP], F32, tag="g48g")
            else:
                fg3 = grp_pool.tile([128, 3 * GRP], F32, tag="fg3")
            if g % (128 // GRP) == 0:
                stage = stage_pool.tile([128, 128], F32, tag="stage")

            o0, o1 = int(goff[g]), int(goff[g + 1])
            gspan = o1 - o0
            lt = lt_pool.tile([KR, TQ * GRP], BF16)
            nc.sync.dma_start(lt[:], lhs_d[:, g * GRP * TQ:(g + 1) * GRP * TQ])
            rtg = rt_pool.tile([KR, gspan_max], BF16, tag="rtg")
            nc.sync.dma_start(rtg[:, 0:gspan], rhs_d[:, o0:o1])
            f2g = f2_pool.tile([128, gspan_max], F32, tag="f2g")
            ck = (gspan + 3) // 4
            for q in range(4):
                a, bb = q * ck, min((q + 1) * ck, gspan)
                if a < bb:
                    nc.sync.dma_start(f2g[:, a:bb],
                                      f2_d[0:1, o0 + a:o0 + bb]
                                      .partition_broadcast(128))

            for j in range(GRP):
                t = g * GRP + j
                cnd = slot_cands[t]
                lo = coff[t] - o0
                pP = ps_P.tile([TQ, 512], F32, tag="pP")
                nc.tensor.matmul(pP[:, 0:cnd], lt[:, j * TQ:(j + 1) * TQ],
                                 rtg[:, lo:lo + cnd], start=True, stop=True)
                nc.vector.max(m8g[:, 8 * j:8 * j + 8], pP[:, 0:cnd])
                if stt_mode:
                    junk = junk_pool.tile([128, 512], F32, tag="junk")
                    for k in range(3):
                        nc.vector.scalar_tensor_tensor(
                            junk[:, 0:cnd], pP[:, 0:cnd],
                            m8g[:, 8 * j + k:8 * j + k + 1],
                            f2g[:, lo:lo + cnd],
                            op0=OP.is_equal, op1=OP.mult,
                            accum_out=fg3[:, 3 * j + k:3 * j + k + 1])
                    continue
                nc.vector.max_index(mig[:, 8 * j:8 * j + 8],
                                    m8g[:, 8 * j:8 * j + 8], pP[:, 0:cnd])
                if j > 0:
                    jp = j - 1
                    tp = g * GRP + jp
                    lop = coff[tp] - o0
                    nc.gpsimd.indirect_copy(g48g[:, 48 * jp:48 * jp + 48],
                                            f2g[:, lop:lop + slot_cands[tp]],
                                            mig[:, 8 * jp:8 * jp + 3],
                                            i_know_ap_gather_is_preferred=True)
            if not stt_mode:
                jp = GRP - 1
                tp = g * GRP + jp
                lop = coff[tp] - o0
                nc.gpsimd.indirect_copy(g48g[:, 48 * jp:48 * jp + 48],
                                        f2g[:, lop:lop + slot_cands[tp]],
                                        mig[:, 8 * jp:8 * jp + 3],
                                        i_know_ap_gather_is_preferred=True)

            # batched tail
            m3 = m8g[:].rearrange("p (j e) -> p j e", e=8)[:, :, 0:3]
            sq1r = sq1_sb[:, g * GRP:(g + 1) * GRP].unsqueeze(-1) \
                                                   .broadcast_to([128, GRP, 3])
            d3 = tail_pool.tile([128, 3 * GRP], F32, tag="d3")
            d3v = d3[:].rearrange("p (j e) -> p j e", e=3)
            nc.vector.tensor_tensor(d3v, sq1r, m3, op=OP.subtract)
            r = tail_pool.tile([128, 3 * GRP], F32, tag="r")
            nc.vector.reciprocal(r[:], d3[:])
            den = tail_pool.tile([128, GRP], F32, tag="den")
            nc.vector.reduce_sum(den[:], r[:].rearrange("p (j e) -> p j e", e=3),
                                 axis=AX.X)
            num = tail_pool.tile([128, GRP], F32, tag="num")
            if stt_mode:
                t1s = tail_pool.tile([128, 3 * GRP], F32, tag="t1s")
                nc.vector.tensor_tensor(t1s[:], r[:], fg3[:], op=OP.mult)
                nc.vector.reduce_sum(num[:],
                                     t1s[:].rearrange("p (j e) -> p j e", e=3),
                                     axis=AX.X)
            else:
                r_rep = r[:].rearrange("p (j e) -> p j e", e=3).unsqueeze(-1) \
                            .broadcast_to([128, GRP, 3, 16])
                g4 = g48g[:].rearrange("p (j k q) -> p j k q", k=3, q=16)
                t1 = tail_pool.tile([128, 48 * GRP], F32, tag="t1")
                t1v = t1[:].rearrange("p (j k q) -> p j k q", k=3, q=16)
                nc.vector.tensor_tensor(t1v, g4, r_rep, op=OP.mult)
                e_rep = E[:].unsqueeze(1).broadcast_to([128, GRP, 48])
                t2 = tail_pool.tile([128, 48 * GRP], F32, tag="t2")
                t2v = t2[:].rearrange("p (j i) -> p j i", i=48)
                nc.vector.tensor_tensor(t2v,
                                        t1[:].rearrange("p (j i) -> p j i", i=48),
                                        e_rep, op=OP.mult)
                nc.vector.reduce_sum(num[:], t2v, axis=AX.X)
            rden = tail_pool.tile([128, GRP], F32, tag="rden")
            nc.vector.reciprocal(rden[:], den[:])
            outv = tail_pool.tile([128, GRP], F32, tag="outv")
            nc.vector.tensor_tensor(outv[:], num[:], rden[:], op=OP.mult)
            col = (g * GRP) % 128
            nc.scalar.activation(stage[:, col:col + GRP], outv[:], ACTF.Sigmoid,
                                 scale=2.0)

            if (g + 1) % (128 // GRP) == 0:
                blk = (g * GRP) // 128
                pT = ps_T.tile([128, 128], F32)
                nc.tensor.transpose(pT[:], stage[:], ident[:])
                oT = stage_pool.tile([128, 128], F32, tag="oT")
                nc.scalar.copy(oT[:], pT[:])
                nc.sync.dma_start(out2d[blk * 128:(blk + 1) * 128, :], oT[:])

    nc.compile()
    return nc


def _split3(v32):
    h = v32.astype(ml_dtypes.bfloat16)
    rr = (v32 - h.astype(np.float32)).astype(np.float32)
    m = rr.astype(ml_dtypes.bfloat16)
    l = (rr - m.astype(np.float32)).astype(ml_dtypes.bfloat16)
    return h, m, l


def _kd_order(pts, leaf):
    idx = np.arange(len(pts))
    out = []
    stack = [idx]
    while stack:
        ids = stack.pop()
        if len(ids) <= leaf:
            out.append(ids)
            continue
        p = pts[ids]
        ax = int(np.argmax(p.max(0) - p.min(0)))
        k = (len(ids) // 2 // leaf) * leaf
        if k == 0:
            k = len(ids) // 2
        part = np.argpartition(p[:, ax], k)
        stack.append(ids[part[k:]])
        stack.append(ids[part[:k]])
    return np.concatenate(out)


def _prep_core(xyz1h, xyz2b, f2row):
    """Returns dict with order, per-tile candidate index lists, counts."""
    q = xyz1h.T.astype(np.float64)          # [NQ, 3]
    r = xyz2b.T.astype(np.float64)          # [S, 3]
    order = _kd_order(q, SUB)
    qs = q[order]
    nsub = NQ // SUB
    qsub = qs.reshape(nsub, SUB, 3)
    c = qsub.mean(1)
    rho = np.sqrt(((qsub - c[:, None, :]) ** 2).sum(2)).max(1)
    dc = np.sqrt(((c[:, None, :] - r[None, :, :]) ** 2).sum(2))
    d3c = np.partition(dc, 2, axis=1)[:, 2]
    need = dc <= (d3c + 2 * rho + 1e-3)[:, None]
    need_t = need.reshape(NT, TQ // SUB, S).any(1)      # [NT, S]
    counts = need_t.sum(1)
    padded = np.maximum(PAD, ((counts + PAD - 1) // PAD) * PAD)
    tile_rank = np.argsort(-padded, kind='stable')       # slot -> original tile
    return {"order": order, "need_t": need_t, "padded": padded,
            "tile_rank": tile_rank}


def _make_core_inputs(xyz1h, xyz2b, f2row, prep, slot_cands, goff):
    order = prep["order"]
    need_t = prep["need_t"]
    tile_rank = prep["tile_rank"]

    x1 = xyz1h.astype(np.float32)[:, order]              # permuted queries
    x2 = xyz2b.astype(np.float32)
    f2 = f2row.astype(np.float32).reshape(-1)

    xh, xm, xl = {}, {}, {}
    for cc in range(3):
        xh[cc], xm[cc], xl[cc] = _split3(x1[cc])
    yh, ym, yl = {}, {}, {}
    for cc in range(3):
        yh[cc], ym[cc], yl[cc] = _split3((2.0 * x2[cc]).astype(np.float32))
    sq2 = ((x2[0] * x2[0] + x2[1] * x2[1]) + x2[2] * x2[2]).astype(np.float32)
    sh, sm, sl = _split3(-sq2)

    onesq = np.ones(NQ, ml_dtypes.bfloat16)
    lhs_rows, rhs_rows = [], []
    for cc in range(3):
        lhs_rows.append(xh[cc]); rhs_rows.append(yh[cc])
    lhs_rows.append(onesq); rhs_rows.append(sh)
    for cc in range(3):
        lhs_rows.append(xh[cc]); rhs_rows.append(ym[cc])
        lhs_rows.append(xm[cc]); rhs_rows.append(yh[cc])
    lhs_rows.append(onesq); rhs_rows.append(sm)
    for cc in range(3):
        lhs_rows.append(xh[cc]); rhs_rows.append(yl[cc])
        lhs_rows.append(xl[cc]); rhs_rows.append(yh[cc])
        lhs_rows.append(xm[cc]); rhs_rows.append(ym[cc])
    lhs_rows.append(onesq); rhs_rows.append(sl)
    lhs = np.stack(lhs_rows).astype(ml_dtypes.bfloat16)      # [KR, NQ]
    rhs_full = np.stack([np.asarray(rr_, np.float32) for rr_ in rhs_rows]) \
                 .astype(np.float32)                          # [KR, S] fp32 view
    rhs_full_bf = np.stack(rhs_rows).astype(ml_dtypes.bfloat16)

    tot = int(goff[-1])
    rhs_pack = np.zeros((KR, tot), ml_dtypes.bfloat16)
    f2_pack = np.zeros((1, tot), np.float32)
    SQ2H_ROW = 3   # the sh row index
    off = 0
    for slot in range(NT):
        t_orig = tile_rank[slot]
        cidx = np.nonzero(need_t[t_orig])[0]
        w = int(slot_cands[slot])
        assert len(cidx) <= w, (slot, len(cidx), w)
        rhs_pack[:, off:off + len(cidx)] = rhs_full_bf[:, cidx]
        if len(cidx) < w:
            rhs_pack[SQ2H_ROW, off + len(cidx):off + w] = \
                ml_dtypes.bfloat16(-1e30)
        f2_pack[0, off:off + len(cidx)] = f2[cidx]
        off += w
    assert off == tot

    # lhs permuted additionally by tile rank: slot s covers original tile
    # tile_rank[s], i.e. queries order[tile_rank[s]*128 : +128]
    qsel = np.concatenate([np.arange(tile_rank[s] * TQ, tile_rank[s] * TQ + TQ)
                           for s in range(NT)])
    lhs_slot = np.ascontiguousarray(lhs[:, qsel])

    sq1 = ((x1[0] * x1[0] + x1[1] * x1[1]) + x1[2] * x1[2]).astype(np.float32)
    sq1e = (sq1 + np.float32(EPS)).astype(np.float32)[qsel]
    sq1_t = np.ascontiguousarray(sq1e.reshape(NT, TQ).T)

    # final query permutation: device position i corresponds to
    # original query order[qsel[i]]
    perm = order[qsel]

    return {
        "lhs_d": lhs_slot,
        "rhs_d": np.ascontiguousarray(rhs_pack),
        "f2_d": np.ascontiguousarray(f2_pack),
        "sq1_d": sq1_t,
    }, perm


def kernel(xyz1, xyz2, points2):
    xyz1 = np.ascontiguousarray(np.asarray(xyz1, dtype=np.float32))
    xyz2 = np.ascontiguousarray(np.asarray(xyz2, dtype=np.float32))
    points2 = np.ascontiguousarray(np.asarray(points2, dtype=np.float32))

    preps = []
    for c in range(N_CORES):
        b, h = c // 2, c % 2
        preps.append(_prep_core(xyz1[b][:, h * NQ:(h + 1) * NQ],
                                xyz2[b], points2[b]))
    padded_sorted = np.stack([np.sort(p["padded"])[::-1] for p in preps])
    slot_cands = padded_sorted.max(0).astype(np.int64)       # [NT]
    goff = np.zeros(NT // GRP + 1, np.int64)
    for g in range(NT // GRP):
        goff[g + 1] = goff[g] + slot_cands[g * GRP:(g + 1) * GRP].sum()
    gspan_max = int(max(goff[g + 1] - goff[g] for g in range(NT // GRP)))

    key = tuple(slot_cands.tolist())
    if key not in _cache:
        _cache[key] = build_nc([int(x) for x in slot_cands], goff, gspan_max)
    nc = _cache[key]

    in_maps, perms = [], []
    for c in range(N_CORES):
        b, h = c // 2, c % 2
        im, perm = _make_core_inputs(xyz1[b][:, h * NQ:(h + 1) * NQ],
                                     xyz2[b], points2[b],
                                     preps[c], slot_cands, goff)
        in_maps.append(im)
        perms.append(perm)

    res = run_bass_kernel_spmd(nc, in_maps, core_ids=list(range(N_CORES)),
                               trace=TRACE)
    if TRACE:
        _cache["last_exec_time_ns"] = res.exec_time_ns
    out = np.empty((B, N), dtype=np.float32)
    for c in range(N_CORES):
        b, h = c // 2, c % 2
        seg = np.empty(NQ, np.float32)
        seg[perms[c]] = res.results[c]["outc"]
        out[b, h * NQ:(h + 1) * NQ] = seg
    return out


if __name__ == "__main__":
    rng = np.random.default_rng(0)
    xyz1 = rng.standard_normal((B, 3, N)).astype(np.float32)
    xyz2 = rng.standard_normal((B, 3, S)).astype(np.float32)
    points2 = rng.standard_normal((B, 1, S)).astype(np.float32)
    out = kernel(xyz1, xyz2, points2)
    print(out.shape, out[0, :5])
